# revision 15
# baseline (speedup 1.0000x reference)
"""Trainium2 Bass kernel for nn_Net_44349832298833 (vq_codebook).

Math: the reference's 10-iteration loop collapses algebraically. With
  e   = embed_w[argmax_k (xs_pad_in @ (embed_w*inv_norm).T)] @ W + b
  t0  = mean(xs_pad_out, axis=-2)
  v   = valid mask from ilens
  A   = sum(v*|e|^2),  B = sum(v*e.t0),  C = sum(v*|t0|^2)
the loss is  38.5*A - 11*B + C   (sum_{i=1..10} ((i*e - t0)^2 * v) / 10).

Device work per token: score matmul (K=1000) + argmax -> onehot,
t0 via block-ones matmul over the 10 xs_pad_out slices, then
G[d,k] = sum_t v*t0[t,d]*onehot[t,k] and c[k] = sum_t v*onehot[t,k]
accumulated in PSUM across all tiles. Host folds weights (EWb = E@W+b)
and combines: A = (c/s)@|EWb|^2-rows, B = sum(EWb.T * G), C from device.

Sharding: data-parallel over B across 8 cores, batches greedily balanced
by sum(ilens); only the valid token prefixes are shipped (packed dense),
in bf16. Same program on all cores (token count padded to a common tile
count); padding tokens carry a zero mask so they contribute nothing.
"""
import sys
import numpy as np

sys.path.insert(0, "/opt/trn_rl_repo")

import ml_dtypes
import concourse.bass as bass
import concourse.tile as tile
from concourse import mybir
from concourse.bass_utils import run_bass_kernel_spmd

F32 = mybir.dt.float32
BF16 = mybir.dt.bfloat16
BF = ml_dtypes.bfloat16
ACTF = mybir.ActivationFunctionType
ALU = mybir.AluOpType

B, T, D, TNUM, K = 32, 512, 256, 10, 1000
NCORES = 8
TOK = 120          # tokens per tile
GRP = 12           # tokens per t0-matmul group (12*10 = 120 partitions)
NG = TOK // GRP    # 10 groups per tile
TPG = 2            # tiles per supertile (DMA batching)
KC = 500           # score/G k-chunk (psum bank = 500 f32)
MEAN = np.float32(1.0 / TNUM)


def _split_multi_waits(nc, max_waits=1):
    # This container's walrus supports one sync-wait per instruction;
    # hoist extras into standalone NoOps on the same engine.
    n = 0
    for fn in nc.m.functions:
        for bb in fn.blocks:
            out = []
            for inst in bb.instructions:
                si = inst.sync_info
                if si is not None and si.on_wait and len(si.on_wait) > max_waits:
                    for w in list(si.on_wait)[:-max_waits]:
                        out.append(mybir.InstNoOp(
                            name=f"waitsplit-{nc.next_id()}",
                            sync_info=mybir.SyncInfo(on_wait=[w], on_update=[]),
                            bass_nofuse=True,
                            engine=inst.engine,
                        ))
                        n += 1
                    si.on_wait = list(si.on_wait)[-max_waits:]
                out.append(inst)
            bb.instructions = out
    return n


def build_nc(n_tiles: int, iters: int = 1, split_waits: bool = True,
             tpg: int = TPG, xs_bufs: int = 2, work_bufs: int = 3):
    """One SPMD program for all 8 cores, n_tiles tiles of TOK tokens each.
    iters>1 repeats the whole computation (for wall-clock benchmarking)."""
    assert n_tiles % tpg == 0
    nst = n_tiles // tpg
    ntok = n_tiles * TOK

    nc = bass.Bass("TRN2", target_bir_lowering=False, debug=False)
    xs = nc.dram_tensor("xs", [ntok * TNUM, D], BF16, kind="ExternalInput").ap()
    xt = nc.dram_tensor("xt", [D, ntok], BF16, kind="ExternalInput").ap()
    ets = nc.dram_tensor("ets", [D, K], BF16, kind="ExternalInput").ap()
    lmap = nc.dram_tensor("lmap", [TOK, NG * TOK], BF16, kind="ExternalInput").ap()
    mkf = nc.dram_tensor("mkf", [128, n_tiles], F32, kind="ExternalInput").ap()
    g_out = nc.dram_tensor("g_out", [128, 2 * K], BF16, kind="ExternalOutput").ap()
    c_out = nc.dram_tensor("c_out", [1, K], F32, kind="ExternalOutput").ap()
    cc_out = nc.dram_tensor("cc_out", [128, 1], F32, kind="ExternalOutput").ap()

    # DRAM views
    xs_v = xs.rearrange("(st jj g p) d -> st p jj g d", st=nst, jj=tpg, g=NG, p=TOK)
    xt_v = xt.rearrange("(dc p) (st tt) -> st p dc tt", dc=2, p=128, st=nst)
    ets_v = ets.rearrange("(dc p) k -> p dc k", dc=2)

    with tile.TileContext(nc) as tc:
        with tc.tile_pool(name="const", bufs=1) as constp, \
             tc.tile_pool(name="xsp", bufs=xs_bufs) as xsp, \
             tc.tile_pool(name="xtp", bufs=2) as xtp, \
             tc.tile_pool(name="work", bufs=work_bufs) as work, \
             tc.tile_pool(name="acc", bufs=1) as accp, \
             tc.tile_pool(name="gps", bufs=1, space="PSUM") as gps, \
             tc.tile_pool(name="cps", bufs=1, space="PSUM") as cps, \
             tc.tile_pool(name="tps", bufs=2, space="PSUM") as tps:

            # warm the ACT function tables while DMAs run
            warm = constp.tile([1, 1], F32, tag="warm")
            nc.vector.memset(warm[:], 0.0)
            warm2 = constp.tile([1, 1], BF16, tag="warm2")
            nc.scalar.activation(warm2[:], warm[:], ACTF.Copy)

            # constants on the ACT HWDGE ring; xs stream owns the SP ring
            ets_sb = constp.tile([128, 2, K], BF16, tag="ets")
            lmap_sb = constp.tile([TOK, NG * TOK], BF16, tag="lmap")
            mkf_sb = constp.tile([128, n_tiles], F32, tag="mkf")
            mkh_sb = constp.tile([128, n_tiles], BF16, tag="mkh")
            nc.scalar.dma_start(ets_sb[:], ets_v)
            nc.scalar.dma_start(lmap_sb[:], lmap)
            nc.scalar.dma_start(mkf_sb[:], mkf)
            nc.vector.tensor_copy(mkh_sb[:], mkf_sb[:])  # bf16(0.1*v)

            for it in range(iters):
                g_ps = [[gps.tile([128, KC], F32, tag=f"g{dc}{kc}", name=f"g{dc}{kc}")
                         for kc in range(2)] for dc in range(2)]
                c_ps = [cps.tile([1, KC], F32, tag=f"c{kc}", name=f"c{kc}") for kc in range(2)]
                rc = accp.tile([128, n_tiles], F32, tag="rc")
                nc.vector.memset(rc[:], 0.0)

                for st in range(nst):
                    xs_sb = xsp.tile([TOK, tpg, NG, D], BF16, tag="xs")
                    nc.sync.dma_start(xs_sb[:], xs_v[st])
                    xt_sb = xtp.tile([128, 2, tpg * TOK], BF16, tag="xt")
                    nc.scalar.dma_start(xt_sb[:], xt_v[st])

                    for jj in range(tpg):
                        j = st * tpg + jj
                        first = j == 0
                        last = j == n_tiles - 1

                        # score chunks -> psum, drained to SBUF by ACT fast
                        # so the transient psum slots never wait on DVE
                        s_ps = [tps.tile([TOK, KC], F32, tag="t", name=f"s{i}") for i in range(2)]
                        for kc in range(2):
                            for dc in range(2):
                                nc.tensor.matmul(
                                    s_ps[kc][:],
                                    xt_sb[:, dc, jj * TOK:(jj + 1) * TOK],
                                    ets_sb[:, dc, kc * KC:(kc + 1) * KC],
                                    start=(dc == 0), stop=(dc == 1))
                        sc_sb = work.tile([TOK, 2 * KC], F32, tag="sc")
                        for kc in range(2):
                            nc.scalar.activation(sc_sb[:, kc * KC:(kc + 1) * KC],
                                                 s_ps[kc][:], ACTF.Copy)

                        # t0 = sum_i xs[t, i, :] via block-ones matmuls
                        t0_ps = tps.tile([TOK, D], F32, tag="t")
                        for g in range(NG):
                            nc.tensor.matmul(
                                t0_ps[:],
                                lmap_sb[:, g * TOK:(g + 1) * TOK],
                                xs_sb[:, jj, g, :],
                                start=(g == 0), stop=(g == NG - 1))
                        # t0v = (0.1 * v) * t0  (psum -> sbuf bf16)
                        t0v = work.tile([TOK, D], BF16, tag="t0v")
                        nc.scalar.activation(t0v[:], t0_ps[:], ACTF.Copy,
                                             scale=mkf_sb[0:TOK, j:j + 1])

                        # row max and onehot = (score == max) -> bf16, from SBUF
                        mm = work.tile([TOK, 1], F32, tag="mm")
                        nc.vector.tensor_reduce(mm[:], sc_sb[:],
                                                axis=mybir.AxisListType.X, op=ALU.max)
                        oh = work.tile([TOK, 2 * KC], BF16, tag="oh")
                        nc.vector.tensor_scalar(oh[:], sc_sb[:],
                                                mm[:, 0:1], None, op0=ALU.is_equal)

                        # G[dc][kc] += t0v.T-chunk x onehot ; c[kc] += v x onehot
                        for kc in range(2):
                            for dc in range(2):
                                nc.tensor.matmul(
                                    g_ps[dc][kc][:],
                                    t0v[:, dc * 128:(dc + 1) * 128],
                                    oh[:, kc * KC:(kc + 1) * KC],
                                    start=first, stop=last)
                            nc.tensor.matmul(
                                c_ps[kc][:],
                                mkh_sb[0:TOK, j:j + 1],
                                oh[:, kc * KC:(kc + 1) * KC],
                                start=first, stop=last)

                        # C: accumulate |t0v|^2 per partition into rc[:, j]
                        sq = work.tile([TOK, D], BF16, tag="sq")
                        nc.vector.scalar_tensor_tensor(
                            sq[:], t0v[:], 1.0, t0v[:],
                            op0=ALU.bypass, op1=ALU.mult,
                            accum_out=rc[0:TOK, j:j + 1])

                # outputs (overwritten every iter; identical data).
                # psum drains split across ACT and DVE; G shipped as bf16.
                g_sb = accp.tile([128, 2 * K], BF16, tag="gsb")
                for dc in range(2):
                    dst = g_sb[:, dc * K: dc * K + KC]
                    dst2 = g_sb[:, dc * K + KC: (dc + 1) * K]
                    nc.scalar.activation(dst, g_ps[dc][0][:], ACTF.Copy)
                    nc.vector.tensor_copy(dst2, g_ps[dc][1][:])
                c_sb = accp.tile([1, K], F32, tag=f"csb_{it}")
                nc.scalar.activation(c_sb[:, 0:KC], c_ps[0][:], ACTF.Copy)
                nc.vector.tensor_copy(c_sb[:, KC:K], c_ps[1][:])
                cc_sb = accp.tile([128, 1], F32, tag=f"ccsb_{it}")
                nc.vector.tensor_reduce(cc_sb[:], rc[:],
                                        axis=mybir.AxisListType.X, op=ALU.add)
                nc.scalar.dma_start(g_out, g_sb[:])
                nc.scalar.dma_start(c_out, c_sb[:])
                nc.scalar.dma_start(cc_out, cc_sb[:])

    if split_waits:
        _split_multi_waits(nc)
    return nc


def _build_lmap():
    lm = np.zeros((TOK, NG * TOK), dtype=BF)
    for g in range(NG):
        for p in range(TOK):
            lm[p, g * TOK + g * GRP + p // TNUM] = 1
    return lm


def pack_inputs(xs_pad_in, xs_pad_out, ilens, embed_w):
    ilens = np.asarray(ilens).astype(np.int64)
    ilens = np.minimum(np.maximum(ilens, 0), T)
    order = np.argsort(-ilens, kind="stable")
    assign = [[] for _ in range(NCORES)]
    loads = np.zeros(NCORES, dtype=np.int64)
    for bi in order:
        c = int(np.argmin(loads))
        assign[c].append(int(bi))
        loads[c] += int(ilens[bi])
    n_tiles = int(max(1, -(-int(loads.max()) // TOK)))
    n_tiles = -(-n_tiles // TPG) * TPG  # multiple of TPG (2) and of any tpg that divides it
    ntok = n_tiles * TOK

    inv = 1.0 / np.linalg.norm(np.asarray(embed_w, np.float32), axis=1)
    ets_np = (np.asarray(embed_w, np.float32) * inv[:, None]).T  # [D, K]
    ets_np = np.ascontiguousarray(ets_np).astype(BF)
    lmap_np = _build_lmap()

    xs_in = np.asarray(xs_pad_in, np.float32)
    xs_out = np.asarray(xs_pad_out, np.float32)

    in_maps = []
    for c in range(NCORES):
        nval = int(loads[c])
        xsb = np.zeros((ntok, TNUM, D), dtype=BF)
        xb = np.zeros((ntok, D), dtype=np.float32)
        pos = 0
        for bi in assign[c]:
            L = int(ilens[bi])
            if L > 0:
                xsb[pos:pos + L] = xs_out[bi, :L]
                xb[pos:pos + L] = xs_in[bi, :L]
                pos += L
        mk = np.zeros((128, n_tiles), dtype=np.float32)
        for j in range(n_tiles):
            lo = j * TOK
            n = min(max(nval - lo, 0), TOK)
            mk[:n, j] = float(MEAN)
        in_maps.append({
            "xs": xsb.reshape(ntok * TNUM, D),
            "xt": np.ascontiguousarray(xb.T).astype(BF),
            "ets": ets_np,
            "lmap": lmap_np,
            "mkf": mk,
        })
    return in_maps, n_tiles


_NC_CACHE = {}


def run_cores(in_maps, n_tiles, iters=1):
    key = (n_tiles, iters)
    if key not in _NC_CACHE:
        _NC_CACHE[key] = build_nc(n_tiles, iters)
    nc = _NC_CACHE[key]
    return run_bass_kernel_spmd(nc, in_maps, core_ids=list(range(NCORES)))


def combine(results, embed_w, W, b):
    E = np.asarray(embed_w, np.float64)
    Wf = np.asarray(W, np.float64)
    bf = np.asarray(b, np.float64)
    EWb = E @ Wf + bf                    # [K, D]
    s2 = (EWb * EWb).sum(axis=1)         # [K]
    sc = float(np.float32(MEAN).astype(BF))  # bf16(0.1) as used on device
    loss = 0.0
    for r in results:
        G = r["g_out"].astype(np.float64).reshape(128, 2, K)  # [p, dc, k]
        c = r["c_out"].astype(np.float64)[0] / sc             # counts
        C = r["cc_out"].astype(np.float64).sum()
        A = float(c @ s2)
        Bv = float((EWb[:, :128].T * G[:, 0, :]).sum()
                   + (EWb[:, 128:].T * G[:, 1, :]).sum())
        loss += 38.5 * A - 11.0 * Bv + C
    return np.asarray(loss, dtype=np.float32)


def kernel(xs_pad_in, xs_pad_out, ilens, ys_pad=None, embed_w=None, W=None, b=None):
    in_maps, n_tiles = pack_inputs(xs_pad_in, xs_pad_out, ilens, embed_w)
    res = run_cores(in_maps, n_tiles)
    return combine(res.results, embed_w, W, b)


# revision 17
# speedup vs baseline: 801.6076x; 801.6076x over previous
"""Trainium2 Bass kernel for nn_Net_44349832298833 (vq_codebook).

Math: the reference's 10-iteration loop collapses algebraically. With
  e   = embed_w[argmax_k (xs_pad_in @ (embed_w*inv_norm).T)] @ W + b
  t0  = mean(xs_pad_out, axis=-2)
  v   = valid mask from ilens
  A   = sum(v*|e|^2),  B = sum(v*e.t0),  C = sum(v*|t0|^2)
the loss is  38.5*A - 11*B + C   (sum_{i=1..10} ((i*e - t0)^2 * v) / 10).

Device work per token: score matmul (K=1000) + argmax -> onehot,
t0 via block-ones matmul over the 10 xs_pad_out slices, then
G[d,k] = sum_t v*t0[t,d]*onehot[t,k] and c[k] = sum_t v*onehot[t,k]
accumulated in PSUM across all tiles. Host folds weights (EWb = E@W+b)
and combines: A = (c/s)@|EWb|^2-rows, B = sum(EWb.T * G), C from device.

Sharding: data-parallel over B across 8 cores, batches greedily balanced
by sum(ilens); only the valid token prefixes are shipped (packed dense),
in bf16. Same program on all cores (token count padded to a common tile
count); padding tokens carry a zero mask so they contribute nothing.
"""
import sys
import numpy as np

sys.path.insert(0, "/opt/trn_rl_repo")

import ml_dtypes
import concourse.bass as bass
import concourse.tile as tile
from concourse import mybir
from concourse.bass_utils import run_bass_kernel_spmd

F32 = mybir.dt.float32
BF16 = mybir.dt.bfloat16
BF = ml_dtypes.bfloat16
ACTF = mybir.ActivationFunctionType
ALU = mybir.AluOpType

B, T, D, TNUM, K = 32, 512, 256, 10, 1000
NCORES = 8
TOK = 120          # tokens per tile
GRP = 12           # tokens per t0-matmul group (12*10 = 120 partitions)
NG = TOK // GRP    # 10 groups per tile
TPG = 2            # tiles per supertile (DMA batching)
KC = 500           # score/G k-chunk (psum bank = 500 f32)
MEAN = np.float32(1.0 / TNUM)


def _split_multi_waits(nc, max_waits=1):
    # This container's walrus supports one sync-wait per instruction;
    # hoist extras into standalone NoOps on the same engine.
    n = 0
    for fn in nc.m.functions:
        for bb in fn.blocks:
            out = []
            for inst in bb.instructions:
                si = inst.sync_info
                if si is not None and si.on_wait and len(si.on_wait) > max_waits:
                    for w in list(si.on_wait)[:-max_waits]:
                        out.append(mybir.InstNoOp(
                            name=f"waitsplit-{nc.next_id()}",
                            sync_info=mybir.SyncInfo(on_wait=[w], on_update=[]),
                            bass_nofuse=True,
                            engine=inst.engine,
                        ))
                        n += 1
                    si.on_wait = list(si.on_wait)[-max_waits:]
                out.append(inst)
            bb.instructions = out
    return n


def build_nc(n_tiles: int, iters: int = 1, split_waits: bool = True,
             tpg: int = TPG, xs_bufs: int = 2, work_bufs: int = 3):
    """One SPMD program for all 8 cores, n_tiles tiles of TOK tokens each.
    iters>1 repeats the whole computation (for wall-clock benchmarking)."""
    assert n_tiles % tpg == 0
    nst = n_tiles // tpg
    ntok = n_tiles * TOK

    nc = bass.Bass("TRN2", target_bir_lowering=False, debug=False)
    xs = nc.dram_tensor("xs", [ntok * TNUM, D], BF16, kind="ExternalInput").ap()
    xt = nc.dram_tensor("xt", [D, ntok], BF16, kind="ExternalInput").ap()
    ets = nc.dram_tensor("ets", [D, K], BF16, kind="ExternalInput").ap()
    lmap = nc.dram_tensor("lmap", [TOK, NG * TOK], BF16, kind="ExternalInput").ap()
    mkf = nc.dram_tensor("mkf", [128, n_tiles], F32, kind="ExternalInput").ap()
    g_out = nc.dram_tensor("g_out", [128, 2 * K], BF16, kind="ExternalOutput").ap()
    c_out = nc.dram_tensor("c_out", [1, K], F32, kind="ExternalOutput").ap()
    cc_out = nc.dram_tensor("cc_out", [128, 1], F32, kind="ExternalOutput").ap()

    # DRAM views
    xs_v = xs.rearrange("(st jj g p) d -> st p jj g d", st=nst, jj=tpg, g=NG, p=TOK)
    xt_v = xt.rearrange("(dc p) (st tt) -> st p dc tt", dc=2, p=128, st=nst)
    ets_v = ets.rearrange("(dc p) k -> p dc k", dc=2)

    with tile.TileContext(nc) as tc:
        with tc.tile_pool(name="const", bufs=1) as constp, \
             tc.tile_pool(name="xsp", bufs=xs_bufs) as xsp, \
             tc.tile_pool(name="xtp", bufs=2) as xtp, \
             tc.tile_pool(name="work", bufs=work_bufs) as work, \
             tc.tile_pool(name="acc", bufs=1) as accp, \
             tc.tile_pool(name="gps", bufs=1, space="PSUM") as gps, \
             tc.tile_pool(name="cps", bufs=1, space="PSUM") as cps, \
             tc.tile_pool(name="tps", bufs=2, space="PSUM") as tps:

            # warm the ACT function tables while DMAs run
            warm = constp.tile([1, 1], F32, tag="warm")
            nc.vector.memset(warm[:], 0.0)
            warm2 = constp.tile([1, 1], BF16, tag="warm2")
            nc.scalar.activation(warm2[:], warm[:], ACTF.Copy)

            # constants on the ACT HWDGE ring; xs stream owns the SP ring
            ets_sb = constp.tile([128, 2, K], BF16, tag="ets")
            lmap_sb = constp.tile([TOK, NG * TOK], BF16, tag="lmap")
            mkf_sb = constp.tile([128, n_tiles], F32, tag="mkf")
            mkh_sb = constp.tile([128, n_tiles], BF16, tag="mkh")
            nc.scalar.dma_start(ets_sb[:], ets_v)
            nc.scalar.dma_start(lmap_sb[:], lmap)
            nc.scalar.dma_start(mkf_sb[:], mkf)
            nc.vector.tensor_copy(mkh_sb[:], mkf_sb[:])  # bf16(0.1*v)

            for it in range(iters):
                g_ps = [[gps.tile([128, KC], F32, tag=f"g{dc}{kc}", name=f"g{dc}{kc}")
                         for kc in range(2)] for dc in range(2)]
                c_ps = [cps.tile([1, KC], F32, tag=f"c{kc}", name=f"c{kc}") for kc in range(2)]
                rc = accp.tile([128, n_tiles], F32, tag="rc")
                nc.vector.memset(rc[:], 0.0)

                for st in range(nst):
                    xs_sb = xsp.tile([TOK, tpg, NG, D], BF16, tag="xs")
                    if st == 0:
                        # split the first load so tile 0 can start sooner
                        for jj in range(tpg):
                            nc.sync.dma_start(xs_sb[:, jj:jj + 1],
                                              xs_v[st][:, jj:jj + 1])
                    else:
                        nc.sync.dma_start(xs_sb[:], xs_v[st])
                    xt_sb = xtp.tile([128, 2, tpg * TOK], BF16, tag="xt")
                    nc.scalar.dma_start(xt_sb[:], xt_v[st])

                    for jj in range(tpg):
                        j = st * tpg + jj
                        first = j == 0
                        last = j == n_tiles - 1

                        # score chunks -> psum, drained to SBUF by ACT fast
                        # so the transient psum slots never wait on DVE
                        s_ps = [tps.tile([TOK, KC], F32, tag="t", name=f"s{i}") for i in range(2)]
                        for kc in range(2):
                            for dc in range(2):
                                nc.tensor.matmul(
                                    s_ps[kc][:],
                                    xt_sb[:, dc, jj * TOK:(jj + 1) * TOK],
                                    ets_sb[:, dc, kc * KC:(kc + 1) * KC],
                                    start=(dc == 0), stop=(dc == 1))
                        sc_sb = work.tile([TOK, 2 * KC], F32, tag="sc")
                        for kc in range(2):
                            nc.scalar.activation(sc_sb[:, kc * KC:(kc + 1) * KC],
                                                 s_ps[kc][:], ACTF.Copy)

                        # t0 = sum_i xs[t, i, :] via block-ones matmuls
                        t0_ps = tps.tile([TOK, D], F32, tag="t")
                        for g in range(NG):
                            nc.tensor.matmul(
                                t0_ps[:],
                                lmap_sb[:, g * TOK:(g + 1) * TOK],
                                xs_sb[:, jj, g, :],
                                start=(g == 0), stop=(g == NG - 1))
                        # t0v = (0.1 * v) * t0  (psum -> sbuf bf16)
                        t0v = work.tile([TOK, D], BF16, tag="t0v")
                        nc.scalar.activation(t0v[:], t0_ps[:], ACTF.Copy,
                                             scale=mkf_sb[0:TOK, j:j + 1])

                        # row max and onehot = (score == max) -> bf16, from SBUF
                        mm = work.tile([TOK, 1], F32, tag="mm")
                        nc.vector.tensor_reduce(mm[:], sc_sb[:],
                                                axis=mybir.AxisListType.X, op=ALU.max)
                        oh = work.tile([TOK, 2 * KC], BF16, tag="oh")
                        nc.vector.tensor_scalar(oh[:], sc_sb[:],
                                                mm[:, 0:1], None, op0=ALU.is_equal)

                        # G[dc][kc] += t0v.T-chunk x onehot ; c[kc] += v x onehot
                        for kc in range(2):
                            for dc in range(2):
                                nc.tensor.matmul(
                                    g_ps[dc][kc][:],
                                    t0v[:, dc * 128:(dc + 1) * 128],
                                    oh[:, kc * KC:(kc + 1) * KC],
                                    start=first, stop=last)
                            nc.tensor.matmul(
                                c_ps[kc][:],
                                mkh_sb[0:TOK, j:j + 1],
                                oh[:, kc * KC:(kc + 1) * KC],
                                start=first, stop=last)

                        # C: accumulate |t0v|^2 per partition into rc[:, j]
                        sq = work.tile([TOK, D], BF16, tag="sq")
                        nc.vector.scalar_tensor_tensor(
                            sq[:], t0v[:], 1.0, t0v[:],
                            op0=ALU.bypass, op1=ALU.mult,
                            accum_out=rc[0:TOK, j:j + 1])

                # outputs (overwritten every iter; identical data).
                # psum drains split across ACT and DVE; G shipped as bf16.
                g_sb = accp.tile([128, 2 * K], BF16, tag="gsb")
                for dc in range(2):
                    dst = g_sb[:, dc * K: dc * K + KC]
                    dst2 = g_sb[:, dc * K + KC: (dc + 1) * K]
                    nc.scalar.activation(dst, g_ps[dc][0][:], ACTF.Copy)
                    nc.vector.tensor_copy(dst2, g_ps[dc][1][:])
                    nc.scalar.dma_start(g_out[:, dc * K:(dc + 1) * K],
                                        g_sb[:, dc * K:(dc + 1) * K])
                c_sb = accp.tile([1, K], F32, tag=f"csb_{it}")
                nc.scalar.activation(c_sb[:, 0:KC], c_ps[0][:], ACTF.Copy)
                nc.vector.tensor_copy(c_sb[:, KC:K], c_ps[1][:])
                cc_sb = accp.tile([128, 1], F32, tag=f"ccsb_{it}")
                nc.vector.tensor_reduce(cc_sb[:], rc[:],
                                        axis=mybir.AxisListType.X, op=ALU.add)
                nc.sync.dma_start(c_out, c_sb[:])
                nc.sync.dma_start(cc_out, cc_sb[:])

    if split_waits:
        _split_multi_waits(nc)
    return nc


def _build_lmap():
    lm = np.zeros((TOK, NG * TOK), dtype=BF)
    for g in range(NG):
        for p in range(TOK):
            lm[p, g * TOK + g * GRP + p // TNUM] = 1
    return lm


def pack_inputs(xs_pad_in, xs_pad_out, ilens, embed_w):
    ilens = np.asarray(ilens).astype(np.int64)
    ilens = np.minimum(np.maximum(ilens, 0), T)
    order = np.argsort(-ilens, kind="stable")
    assign = [[] for _ in range(NCORES)]
    loads = np.zeros(NCORES, dtype=np.int64)
    for bi in order:
        c = int(np.argmin(loads))
        assign[c].append(int(bi))
        loads[c] += int(ilens[bi])
    n_tiles = int(max(1, -(-int(loads.max()) // TOK)))
    n_tiles = -(-n_tiles // TPG) * TPG  # multiple of TPG (2) and of any tpg that divides it
    ntok = n_tiles * TOK

    inv = 1.0 / np.linalg.norm(np.asarray(embed_w, np.float32), axis=1)
    ets_np = (np.asarray(embed_w, np.float32) * inv[:, None]).T  # [D, K]
    ets_np = np.ascontiguousarray(ets_np).astype(BF)
    lmap_np = _build_lmap()

    xs_in = np.asarray(xs_pad_in, np.float32)
    xs_out = np.asarray(xs_pad_out, np.float32)

    in_maps = []
    for c in range(NCORES):
        nval = int(loads[c])
        xsb = np.zeros((ntok, TNUM, D), dtype=BF)
        xb = np.zeros((ntok, D), dtype=np.float32)
        pos = 0
        for bi in assign[c]:
            L = int(ilens[bi])
            if L > 0:
                xsb[pos:pos + L] = xs_out[bi, :L]
                xb[pos:pos + L] = xs_in[bi, :L]
                pos += L
        mk = np.zeros((128, n_tiles), dtype=np.float32)
        for j in range(n_tiles):
            lo = j * TOK
            n = min(max(nval - lo, 0), TOK)
            mk[:n, j] = float(MEAN)
        in_maps.append({
            "xs": xsb.reshape(ntok * TNUM, D),
            "xt": np.ascontiguousarray(xb.T).astype(BF),
            "ets": ets_np,
            "lmap": lmap_np,
            "mkf": mk,
        })
    return in_maps, n_tiles


_NC_CACHE = {}


def run_cores(in_maps, n_tiles, iters=1):
    key = (n_tiles, iters)
    if key not in _NC_CACHE:
        _NC_CACHE[key] = build_nc(n_tiles, iters)
    nc = _NC_CACHE[key]
    return run_bass_kernel_spmd(nc, in_maps, core_ids=list(range(NCORES)))


def combine(results, embed_w, W, b):
    E = np.asarray(embed_w, np.float64)
    Wf = np.asarray(W, np.float64)
    bf = np.asarray(b, np.float64)
    EWb = E @ Wf + bf                    # [K, D]
    s2 = (EWb * EWb).sum(axis=1)         # [K]
    sc = float(np.float32(MEAN).astype(BF))  # bf16(0.1) as used on device
    loss = 0.0
    for r in results:
        G = r["g_out"].astype(np.float64).reshape(128, 2, K)  # [p, dc, k]
        c = r["c_out"].astype(np.float64)[0] / sc             # counts
        C = r["cc_out"].astype(np.float64).sum()
        A = float(c @ s2)
        Bv = float((EWb[:, :128].T * G[:, 0, :]).sum()
                   + (EWb[:, 128:].T * G[:, 1, :]).sum())
        loss += 38.5 * A - 11.0 * Bv + C
    return np.asarray(loss, dtype=np.float32)


def kernel(xs_pad_in, xs_pad_out, ilens, ys_pad=None, embed_w=None, W=None, b=None):
    in_maps, n_tiles = pack_inputs(xs_pad_in, xs_pad_out, ilens, embed_w)
    res = run_cores(in_maps, n_tiles)
    return combine(res.results, embed_w, W, b)


# revision 19
# speedup vs baseline: 843.3060x; 1.0520x over previous
"""Trainium2 Bass kernel for nn_Net_44349832298833 (vq_codebook).

Math: the reference's 10-iteration loop collapses algebraically. With
  e   = embed_w[argmax_k (xs_pad_in @ (embed_w*inv_norm).T)] @ W + b
  t0  = mean(xs_pad_out, axis=-2)
  v   = valid mask from ilens
  A   = sum(v*|e|^2),  B = sum(v*e.t0),  C = sum(v*|t0|^2)
the loss is  38.5*A - 11*B + C   (sum_{i=1..10} ((i*e - t0)^2 * v) / 10).

Device work per token: score matmul (K=1000) + argmax -> onehot,
t0 via block-ones matmul over the 10 xs_pad_out slices, then
G[d,k] = sum_t v*t0[t,d]*onehot[t,k] and c[k] = sum_t v*onehot[t,k]
accumulated in PSUM across all tiles. Host folds weights (EWb = E@W+b)
and combines: A = (c/s)@|EWb|^2-rows, B = sum(EWb.T * G), C from device.

Sharding: data-parallel over B across 8 cores, batches greedily balanced
by sum(ilens); only the valid token prefixes are shipped (packed dense),
in bf16. Same program on all cores (token count padded to a common tile
count); padding tokens carry a zero mask so they contribute nothing.
"""
import os
import sys
import numpy as np

for _p in ("/opt/trn_rl_repo", "/root/.axon_site/_ro/trn_rl_repo"):
    if os.path.isdir(_p) and _p not in sys.path:
        sys.path.insert(0, _p)

import ml_dtypes
import concourse.bass as bass
import concourse.tile as tile
from concourse import mybir
from concourse.bass_utils import run_bass_kernel_spmd

F32 = mybir.dt.float32
BF16 = mybir.dt.bfloat16
BF = ml_dtypes.bfloat16
ACTF = mybir.ActivationFunctionType
ALU = mybir.AluOpType

B, T, D, TNUM, K = 32, 512, 256, 10, 1000
NCORES = 8
TOK = 120          # tokens per tile
GRP = 12           # tokens per t0-matmul group (12*10 = 120 partitions)
NG = TOK // GRP    # 10 groups per tile
TPG = 2            # tiles per supertile (DMA batching)
KC = 500           # score/G k-chunk (psum bank = 500 f32)
MEAN = np.float32(1.0 / TNUM)


def _split_multi_waits(nc, max_waits=1):
    # This container's walrus supports one sync-wait per instruction;
    # hoist extras into standalone NoOps on the same engine.
    n = 0
    for fn in nc.m.functions:
        for bb in fn.blocks:
            out = []
            for inst in bb.instructions:
                si = inst.sync_info
                if si is not None and si.on_wait and len(si.on_wait) > max_waits:
                    for w in list(si.on_wait)[:-max_waits]:
                        out.append(mybir.InstNoOp(
                            name=f"waitsplit-{nc.next_id()}",
                            sync_info=mybir.SyncInfo(on_wait=[w], on_update=[]),
                            bass_nofuse=True,
                            engine=inst.engine,
                        ))
                        n += 1
                    si.on_wait = list(si.on_wait)[-max_waits:]
                out.append(inst)
            bb.instructions = out
    return n


def build_nc(n_tiles: int, iters: int = 1, split_waits: bool = True,
             tpg: int = TPG, xs_bufs: int = 2, work_bufs: int = 3):
    """One SPMD program for all 8 cores, n_tiles tiles of TOK tokens each.
    iters>1 repeats the whole computation (for wall-clock benchmarking)."""
    assert n_tiles % tpg == 0
    nst = n_tiles // tpg
    ntok = n_tiles * TOK

    nc = bass.Bass("TRN2", target_bir_lowering=False, debug=False)
    xs = nc.dram_tensor("xs", [ntok * TNUM, D], BF16, kind="ExternalInput").ap()
    xt = nc.dram_tensor("xt", [D, ntok], BF16, kind="ExternalInput").ap()
    ets = nc.dram_tensor("ets", [D, K], BF16, kind="ExternalInput").ap()
    lmap = nc.dram_tensor("lmap", [TOK, NG * TOK], BF16, kind="ExternalInput").ap()
    mkf = nc.dram_tensor("mkf", [128, n_tiles], F32, kind="ExternalInput").ap()
    g_out = nc.dram_tensor("g_out", [128, 2 * K], BF16, kind="ExternalOutput").ap()
    c_out = nc.dram_tensor("c_out", [1, K], F32, kind="ExternalOutput").ap()
    cc_out = nc.dram_tensor("cc_out", [128, 1], F32, kind="ExternalOutput").ap()

    # DRAM views
    xs_v = xs.rearrange("(st jj g p) d -> st p jj g d", st=nst, jj=tpg, g=NG, p=TOK)
    xt_v = xt.rearrange("(dc p) (st tt) -> st p dc tt", dc=2, p=128, st=nst)
    ets_v = ets.rearrange("(dc p) k -> p dc k", dc=2)

    with tile.TileContext(nc) as tc:
        with tc.tile_pool(name="const", bufs=1) as constp, \
             tc.tile_pool(name="xsp", bufs=xs_bufs) as xsp, \
             tc.tile_pool(name="xtp", bufs=2) as xtp, \
             tc.tile_pool(name="work", bufs=work_bufs) as work, \
             tc.tile_pool(name="acc", bufs=1) as accp, \
             tc.tile_pool(name="gps", bufs=1, space="PSUM") as gps, \
             tc.tile_pool(name="cps", bufs=1, space="PSUM") as cps, \
             tc.tile_pool(name="tps", bufs=2, space="PSUM") as tps:

            # warm the ACT function tables while DMAs run
            warm = constp.tile([1, 1], F32, tag="warm")
            nc.vector.memset(warm[:], 0.0)
            warm2 = constp.tile([1, 1], BF16, tag="warm2")
            nc.scalar.activation(warm2[:], warm[:], ACTF.Copy)

            # constants on the ACT HWDGE ring; xs stream owns the SP ring
            ets_sb = constp.tile([128, 2, K], BF16, tag="ets")
            lmap_sb = constp.tile([TOK, NG * TOK], BF16, tag="lmap")
            mkf_sb = constp.tile([128, n_tiles], F32, tag="mkf")
            mkh_sb = constp.tile([128, n_tiles], BF16, tag="mkh")
            nc.scalar.dma_start(ets_sb[:], ets_v)
            nc.scalar.dma_start(lmap_sb[:], lmap)
            nc.scalar.dma_start(mkf_sb[:], mkf)
            nc.vector.tensor_copy(mkh_sb[:], mkf_sb[:])  # bf16(0.1*v)

            for it in range(iters):
                g_ps = [[gps.tile([128, KC], F32, tag=f"g{dc}{kc}", name=f"g{dc}{kc}")
                         for kc in range(2)] for dc in range(2)]
                c_ps = [cps.tile([1, KC], F32, tag=f"c{kc}", name=f"c{kc}") for kc in range(2)]
                rc = accp.tile([128, n_tiles], F32, tag="rc")
                nc.vector.memset(rc[:], 0.0)

                for st in range(nst):
                    xs_sb = xsp.tile([TOK, tpg, NG, D], BF16, tag="xs")
                    xt_sb = xtp.tile([128, 2, tpg * TOK], BF16, tag="xt")
                    if st == 0 and it == 0:
                        # split the first load and slot xt between the
                        # halves so tile 0's score can start ASAP
                        nc.sync.dma_start(xs_sb[:, 0:1], xs_v[st][:, 0:1])
                        nc.sync.dma_start(xt_sb[:], xt_v[st])
                        for jj in range(1, tpg):
                            nc.sync.dma_start(xs_sb[:, jj:jj + 1],
                                              xs_v[st][:, jj:jj + 1])
                    else:
                        nc.sync.dma_start(xs_sb[:], xs_v[st])
                        nc.scalar.dma_start(xt_sb[:], xt_v[st])

                    for jj in range(tpg):
                        j = st * tpg + jj
                        first = j == 0
                        last = j == n_tiles - 1

                        # score chunks -> psum, drained to SBUF by ACT fast
                        # so the transient psum slots never wait on DVE
                        s_ps = [tps.tile([TOK, KC], F32, tag="t", name=f"s{i}") for i in range(2)]
                        for kc in range(2):
                            for dc in range(2):
                                nc.tensor.matmul(
                                    s_ps[kc][:],
                                    xt_sb[:, dc, jj * TOK:(jj + 1) * TOK],
                                    ets_sb[:, dc, kc * KC:(kc + 1) * KC],
                                    start=(dc == 0), stop=(dc == 1))
                        sc_sb = work.tile([TOK, 2 * KC], F32, tag="sc")
                        for kc in range(2):
                            nc.scalar.activation(sc_sb[:, kc * KC:(kc + 1) * KC],
                                                 s_ps[kc][:], ACTF.Copy)

                        # t0 = sum_i xs[t, i, :] via block-ones matmuls
                        t0_ps = tps.tile([TOK, D], F32, tag="t")
                        for g in range(NG):
                            nc.tensor.matmul(
                                t0_ps[:],
                                lmap_sb[:, g * TOK:(g + 1) * TOK],
                                xs_sb[:, jj, g, :],
                                start=(g == 0), stop=(g == NG - 1))
                        # t0v = (0.1 * v) * t0  (psum -> sbuf bf16)
                        t0v = work.tile([TOK, D], BF16, tag="t0v")
                        nc.scalar.activation(t0v[:], t0_ps[:], ACTF.Copy,
                                             scale=mkf_sb[0:TOK, j:j + 1])

                        # row max and onehot = (score == max) -> bf16, from SBUF
                        mm = work.tile([TOK, 1], F32, tag="mm")
                        nc.vector.tensor_reduce(mm[:], sc_sb[:],
                                                axis=mybir.AxisListType.X, op=ALU.max)
                        oh = work.tile([TOK, 2 * KC], BF16, tag="oh")
                        nc.vector.tensor_scalar(oh[:], sc_sb[:],
                                                mm[:, 0:1], None, op0=ALU.is_equal)

                        # G[dc][kc] += t0v.T-chunk x onehot ; c[kc] += v x onehot
                        for kc in range(2):
                            for dc in range(2):
                                nc.tensor.matmul(
                                    g_ps[dc][kc][:],
                                    t0v[:, dc * 128:(dc + 1) * 128],
                                    oh[:, kc * KC:(kc + 1) * KC],
                                    start=first, stop=last)
                            nc.tensor.matmul(
                                c_ps[kc][:],
                                mkh_sb[0:TOK, j:j + 1],
                                oh[:, kc * KC:(kc + 1) * KC],
                                start=first, stop=last)

                        # C: accumulate |t0v|^2 per partition into rc[:, j]
                        sq = work.tile([TOK, D], BF16, tag="sq")
                        nc.vector.scalar_tensor_tensor(
                            sq[:], t0v[:], 1.0, t0v[:],
                            op0=ALU.bypass, op1=ALU.mult,
                            accum_out=rc[0:TOK, j:j + 1])

                # outputs (overwritten every iter; identical data).
                # psum drains split across ACT and DVE; G shipped as bf16.
                g_sb = accp.tile([128, 2 * K], BF16, tag="gsb")
                for dc in range(2):
                    dst = g_sb[:, dc * K: dc * K + KC]
                    dst2 = g_sb[:, dc * K + KC: (dc + 1) * K]
                    nc.scalar.activation(dst, g_ps[dc][0][:], ACTF.Copy)
                    nc.vector.tensor_copy(dst2, g_ps[dc][1][:])
                    nc.scalar.dma_start(g_out[:, dc * K:(dc + 1) * K],
                                        g_sb[:, dc * K:(dc + 1) * K])
                c_sb = accp.tile([1, K], F32, tag=f"csb_{it}")
                nc.scalar.activation(c_sb[:, 0:KC], c_ps[0][:], ACTF.Copy)
                nc.vector.tensor_copy(c_sb[:, KC:K], c_ps[1][:])
                cc_sb = accp.tile([128, 1], F32, tag=f"ccsb_{it}")
                nc.vector.tensor_reduce(cc_sb[:], rc[:],
                                        axis=mybir.AxisListType.X, op=ALU.add)
                nc.sync.dma_start(c_out, c_sb[:])
                nc.sync.dma_start(cc_out, cc_sb[:])

    if split_waits:
        _split_multi_waits(nc)
    return nc


def _build_lmap():
    lm = np.zeros((TOK, NG * TOK), dtype=BF)
    for g in range(NG):
        for p in range(TOK):
            lm[p, g * TOK + g * GRP + p // TNUM] = 1
    return lm


def pack_inputs(xs_pad_in, xs_pad_out, ilens, embed_w):
    ilens = np.asarray(ilens).astype(np.int64)
    ilens = np.minimum(np.maximum(ilens, 0), T)
    order = np.argsort(-ilens, kind="stable")
    assign = [[] for _ in range(NCORES)]
    loads = np.zeros(NCORES, dtype=np.int64)
    for bi in order:
        c = int(np.argmin(loads))
        assign[c].append(int(bi))
        loads[c] += int(ilens[bi])
    n_tiles = int(max(1, -(-int(loads.max()) // TOK)))
    n_tiles = -(-n_tiles // TPG) * TPG  # multiple of TPG (2) and of any tpg that divides it
    ntok = n_tiles * TOK

    inv = 1.0 / np.linalg.norm(np.asarray(embed_w, np.float32), axis=1)
    ets_np = (np.asarray(embed_w, np.float32) * inv[:, None]).T  # [D, K]
    ets_np = np.ascontiguousarray(ets_np).astype(BF)
    lmap_np = _build_lmap()

    xs_in = np.asarray(xs_pad_in, np.float32)
    xs_out = np.asarray(xs_pad_out, np.float32)

    in_maps = []
    for c in range(NCORES):
        nval = int(loads[c])
        xsb = np.zeros((ntok, TNUM, D), dtype=BF)
        xb = np.zeros((ntok, D), dtype=np.float32)
        pos = 0
        for bi in assign[c]:
            L = int(ilens[bi])
            if L > 0:
                xsb[pos:pos + L] = xs_out[bi, :L]
                xb[pos:pos + L] = xs_in[bi, :L]
                pos += L
        mk = np.zeros((128, n_tiles), dtype=np.float32)
        for j in range(n_tiles):
            lo = j * TOK
            n = min(max(nval - lo, 0), TOK)
            mk[:n, j] = float(MEAN)
        in_maps.append({
            "xs": xsb.reshape(ntok * TNUM, D),
            "xt": np.ascontiguousarray(xb.T).astype(BF),
            "ets": ets_np,
            "lmap": lmap_np,
            "mkf": mk,
        })
    return in_maps, n_tiles


_NC_CACHE = {}


def run_cores(in_maps, n_tiles, iters=1):
    key = (n_tiles, iters)
    if key not in _NC_CACHE:
        _NC_CACHE[key] = build_nc(n_tiles, iters)
    nc = _NC_CACHE[key]
    return run_bass_kernel_spmd(nc, in_maps, core_ids=list(range(NCORES)))


def combine(results, embed_w, W, b):
    E = np.asarray(embed_w, np.float64)
    Wf = np.asarray(W, np.float64)
    bf = np.asarray(b, np.float64)
    EWb = E @ Wf + bf                    # [K, D]
    s2 = (EWb * EWb).sum(axis=1)         # [K]
    sc = float(np.float32(MEAN).astype(BF))  # bf16(0.1) as used on device
    loss = 0.0
    for r in results:
        G = r["g_out"].astype(np.float64).reshape(128, 2, K)  # [p, dc, k]
        c = r["c_out"].astype(np.float64)[0] / sc             # counts
        C = r["cc_out"].astype(np.float64).sum()
        A = float(c @ s2)
        Bv = float((EWb[:, :128].T * G[:, 0, :]).sum()
                   + (EWb[:, 128:].T * G[:, 1, :]).sum())
        loss += 38.5 * A - 11.0 * Bv + C
    return np.asarray(loss, dtype=np.float32)


def kernel(xs_pad_in, xs_pad_out, ilens, ys_pad=None, embed_w=None, W=None, b=None):
    in_maps, n_tiles = pack_inputs(xs_pad_in, xs_pad_out, ilens, embed_w)
    res = run_cores(in_maps, n_tiles)
    return combine(res.results, embed_w, W, b)


# revision 27
# speedup vs baseline: 963.7663x; 1.1428x over previous
"""Trainium2 Bass kernel for nn_Net_44349832298833 (vq_codebook).

Math: the reference's 10-iteration loop collapses algebraically. With
  e   = embed_w[argmax_k (xs_pad_in @ (embed_w*inv_norm).T)] @ W + b
  t0  = mean(xs_pad_out, axis=-2)
  v   = valid mask from ilens
  A   = sum(v*|e|^2),  B = sum(v*e.t0),  C = sum(v*|t0|^2)
the loss is  38.5*A - 11*B + C   (sum_{i=1..10} ((i*e - t0)^2 * v) / 10).

Device work per token: score matmul (K=1000) + argmax -> onehot,
t0 via block-ones matmul over the 10 xs_pad_out slices, then
G[d,k] = sum_t v*t0[t,d]*onehot[t,k] and c[k] = sum_t v*onehot[t,k]
accumulated in PSUM across all tiles. Host folds weights (EWb = E@W+b)
and combines: A = (c/s)@|EWb|^2-rows, B = sum(EWb.T * G), C from device.

Sharding: data-parallel over B across 8 cores, batches greedily balanced
by sum(ilens); only the valid token prefixes are shipped (packed dense),
in bf16. Same program on all cores (token count padded to a common tile
count); padding tokens carry a zero mask so they contribute nothing.
"""
import os
import sys
import numpy as np

for _p in ("/opt/trn_rl_repo", "/root/.axon_site/_ro/trn_rl_repo"):
    if os.path.isdir(_p) and _p not in sys.path:
        sys.path.insert(0, _p)

import ml_dtypes
import concourse.bass as bass
import concourse.tile as tile
from concourse import mybir
from concourse.bass_utils import run_bass_kernel_spmd

F32 = mybir.dt.float32
BF16 = mybir.dt.bfloat16
BF = ml_dtypes.bfloat16
ACTF = mybir.ActivationFunctionType
ALU = mybir.AluOpType

B, T, D, TNUM, K = 32, 512, 256, 10, 1000
NCORES = 8
TOK = 120          # tokens per tile
GRP = 12           # tokens per t0-matmul group (12*10 = 120 partitions)
NG = TOK // GRP    # 10 groups per tile
TPG = 2            # tiles per supertile (DMA batching)
KC = 500           # score/G k-chunk (psum bank = 500 f32)
MEAN = np.float32(1.0 / TNUM)


def _split_multi_waits(nc, max_waits=1):
    # This container's walrus supports one sync-wait per instruction;
    # hoist extras into standalone NoOps on the same engine.
    n = 0
    for fn in nc.m.functions:
        for bb in fn.blocks:
            out = []
            for inst in bb.instructions:
                si = inst.sync_info
                if si is not None and si.on_wait and len(si.on_wait) > max_waits:
                    for w in list(si.on_wait)[:-max_waits]:
                        out.append(mybir.InstNoOp(
                            name=f"waitsplit-{nc.next_id()}",
                            sync_info=mybir.SyncInfo(on_wait=[w], on_update=[]),
                            bass_nofuse=True,
                            engine=inst.engine,
                        ))
                        n += 1
                    si.on_wait = list(si.on_wait)[-max_waits:]
                out.append(inst)
            bb.instructions = out
    return n


def build_nc(n_tiles: int, iters: int = 1, split_waits: bool = True,
             tpg: int = TPG, xs_bufs: int = 2, work_bufs: int = 3,
             dve_t0_mod: int = 5, dve_t0_cnt: int = 3):
    """One SPMD program for all 8 cores, n_tiles tiles of TOK tokens each.
    iters>1 repeats the whole computation (for wall-clock benchmarking)."""
    assert n_tiles % tpg == 0
    nst = n_tiles // tpg
    ntok = n_tiles * TOK

    nc = bass.Bass("TRN2", target_bir_lowering=False, debug=False)
    xs = nc.dram_tensor("xs", [ntok * TNUM, D], BF16, kind="ExternalInput").ap()
    xt = nc.dram_tensor("xt", [D, ntok], BF16, kind="ExternalInput").ap()
    ets = nc.dram_tensor("ets", [D, K], BF16, kind="ExternalInput").ap()
    lmap = nc.dram_tensor("lmap", [TOK, NG * TOK], BF16, kind="ExternalInput").ap()
    mkf = nc.dram_tensor("mkf", [128, n_tiles], F32, kind="ExternalInput").ap()
    g_out = nc.dram_tensor("g_out", [128, 2 * K], BF16, kind="ExternalOutput").ap()
    c_out = nc.dram_tensor("c_out", [1, K], F32, kind="ExternalOutput").ap()
    cc_out = nc.dram_tensor("cc_out", [128, 1], F32, kind="ExternalOutput").ap()

    # DRAM views. xs_v: (token,i)-interleaved partitions for the block-ones
    # matmul; xs_tok: token-major partitions (i in free dim) for the DVE
    # pairwise-tree tiles — same bytes, different access pattern.
    xs_v = xs.rearrange("(st jj g p) d -> st p jj g d", st=nst, jj=tpg, g=NG, p=TOK)
    xs_tok = xs.rearrange("(st jj p i) d -> st p jj i d", st=nst, jj=tpg, p=TOK, i=TNUM)
    xt_v = xt.rearrange("(dc p) (st tt) -> st p dc tt", dc=2, p=128, st=nst)
    ets_v = ets.rearrange("(dc p) k -> p dc k", dc=2)

    with tile.TileContext(nc) as tc:
        with tc.tile_pool(name="const", bufs=1) as constp, \
             tc.tile_pool(name="xsp", bufs=xs_bufs) as xsp, \
             tc.tile_pool(name="xtp", bufs=2) as xtp, \
             tc.tile_pool(name="work", bufs=work_bufs) as work, \
             tc.tile_pool(name="acc", bufs=1) as accp, \
             tc.tile_pool(name="gps", bufs=1, space="PSUM") as gps, \
             tc.tile_pool(name="cps", bufs=1, space="PSUM") as cps, \
             tc.tile_pool(name="tps", bufs=2, space="PSUM") as tps:

            # warm the ACT function tables while DMAs run
            warm = constp.tile([1, 1], F32, tag="warm")
            nc.vector.memset(warm[:], 0.0)
            warm2 = constp.tile([1, 1], BF16, tag="warm2")
            nc.scalar.activation(warm2[:], warm[:], ACTF.Copy)

            # constants on the ACT HWDGE ring; xs stream owns the SP ring
            ets_sb = constp.tile([128, 2, K], BF16, tag="ets")
            lmap_sb = constp.tile([TOK, NG * TOK], BF16, tag="lmap")
            mkf_sb = constp.tile([128, n_tiles], F32, tag="mkf")
            mkh_sb = constp.tile([128, n_tiles], BF16, tag="mkh")
            nc.scalar.dma_start(ets_sb[:], ets_v)
            nc.scalar.dma_start(lmap_sb[:], lmap)
            nc.scalar.dma_start(mkf_sb[:], mkf)
            nc.vector.tensor_copy(mkh_sb[:], mkf_sb[:])  # bf16(0.1*v)

            for it in range(iters):
                g_ps = [[gps.tile([128, KC], F32, tag=f"g{dc}{kc}", name=f"g{dc}{kc}")
                         for kc in range(2)] for dc in range(2)]
                c_ps = [cps.tile([1, KC], F32, tag=f"c{kc}", name=f"c{kc}") for kc in range(2)]
                rc = accp.tile([128, n_tiles], F32, tag="rc")
                nc.vector.memset(rc[:], 0.0)

                for st in range(nst):
                    xs_sb = xsp.tile([TOK, tpg, NG, D], BF16, tag="xs")
                    xt_sb = xtp.tile([128, 2, tpg * TOK], BF16, tag="xt")
                    # per-tile loads: tree tiles get the token-major view,
                    # matmul tiles the (token,i)-interleaved view
                    for jj in range(tpg):
                        j = st * tpg + jj
                        view = (xs_tok if j % dve_t0_mod < dve_t0_cnt
                                else xs_v)
                        nc.sync.dma_start(xs_sb[:, jj:jj + 1],
                                          view[st][:, jj:jj + 1])
                        if st == 0 and it == 0 and jj == 0:
                            nc.sync.dma_start(xt_sb[:], xt_v[st])
                    if not (st == 0 and it == 0):
                        nc.scalar.dma_start(xt_sb[:], xt_v[st])

                    for jj in range(tpg):
                        j = st * tpg + jj
                        first = j == 0
                        last = j == n_tiles - 1

                        # score chunks -> psum, drained to SBUF by ACT fast
                        # so the transient psum slots never wait on DVE
                        s_ps = [tps.tile([TOK, KC], F32, tag="t", name=f"s{i}") for i in range(2)]
                        for kc in range(2):
                            for dc in range(2):
                                nc.tensor.matmul(
                                    s_ps[kc][:],
                                    xt_sb[:, dc, jj * TOK:(jj + 1) * TOK],
                                    ets_sb[:, dc, kc * KC:(kc + 1) * KC],
                                    start=(dc == 0), stop=(dc == 1))
                        sc_sb = work.tile([TOK, 2 * KC], F32, tag="sc")
                        for kc in range(2):
                            nc.scalar.activation(sc_sb[:, kc * KC:(kc + 1) * KC],
                                                 s_ps[kc][:], ACTF.Copy)

                        # t0 = sum_i xs[t, i, :]. PE (block-ones matmuls) is
                        # the busiest engine, so a slice of tiles computes it
                        # on DVE instead (bf16 pairwise tree runs at 2x mode)
                        # to balance engine load.
                        t0v = work.tile([TOK, D], BF16, tag="t0v")
                        if j % dve_t0_mod < dve_t0_cnt:
                            tA = work.tile([TOK, 5, D], BF16, tag="tA")
                            nc.vector.tensor_add(tA[:], xs_sb[:, jj, 0:5, :],
                                                 xs_sb[:, jj, 5:10, :])
                            tB = work.tile([TOK, 2, D], BF16, tag="tB")
                            nc.vector.tensor_add(tB[:], tA[:, 0:2, :],
                                                 tA[:, 2:4, :])
                            tC = work.tile([TOK, D], BF16, tag="tC")
                            nc.vector.tensor_add(tC[:], tB[:, 0, :], tB[:, 1, :])
                            tD = work.tile([TOK, D], BF16, tag="tD")
                            nc.vector.tensor_add(tD[:], tC[:], tA[:, 4, :])
                            nc.scalar.activation(t0v[:], tD[:], ACTF.Copy,
                                                 scale=mkf_sb[0:TOK, j:j + 1])
                        else:
                            t0_ps = tps.tile([TOK, D], F32, tag="t")
                            for g in range(NG):
                                nc.tensor.matmul(
                                    t0_ps[:],
                                    lmap_sb[:, g * TOK:(g + 1) * TOK],
                                    xs_sb[:, jj, g, :],
                                    start=(g == 0), stop=(g == NG - 1))
                            # t0v = (0.1 * v) * t0  (psum -> sbuf bf16)
                            nc.scalar.activation(t0v[:], t0_ps[:], ACTF.Copy,
                                                 scale=mkf_sb[0:TOK, j:j + 1])

                        # row max and onehot = (score == max) -> bf16, from SBUF
                        mm = work.tile([TOK, 1], F32, tag="mm")
                        nc.vector.tensor_reduce(mm[:], sc_sb[:],
                                                axis=mybir.AxisListType.X, op=ALU.max)
                        oh = work.tile([TOK, 2 * KC], BF16, tag="oh")
                        nc.vector.tensor_scalar(oh[:], sc_sb[:],
                                                mm[:, 0:1], None, op0=ALU.is_equal)

                        # G[dc][kc] += t0v.T-chunk x onehot ; c[kc] += v x onehot
                        for kc in range(2):
                            for dc in range(2):
                                nc.tensor.matmul(
                                    g_ps[dc][kc][:],
                                    t0v[:, dc * 128:(dc + 1) * 128],
                                    oh[:, kc * KC:(kc + 1) * KC],
                                    start=first, stop=last)
                            nc.tensor.matmul(
                                c_ps[kc][:],
                                mkh_sb[0:TOK, j:j + 1],
                                oh[:, kc * KC:(kc + 1) * KC],
                                start=first, stop=last)

                        # C: accumulate |t0v|^2 per partition into rc[:, j]
                        sq = work.tile([TOK, D], BF16, tag="sq")
                        nc.vector.scalar_tensor_tensor(
                            sq[:], t0v[:], 1.0, t0v[:],
                            op0=ALU.bypass, op1=ALU.mult,
                            accum_out=rc[0:TOK, j:j + 1])

                # outputs (overwritten every iter; identical data).
                # psum drains split across ACT and DVE; G shipped as bf16.
                g_sb = accp.tile([128, 2 * K], BF16, tag="gsb")
                for dc in range(2):
                    dst = g_sb[:, dc * K: dc * K + KC]
                    dst2 = g_sb[:, dc * K + KC: (dc + 1) * K]
                    nc.scalar.activation(dst, g_ps[dc][0][:], ACTF.Copy)
                    nc.vector.tensor_copy(dst2, g_ps[dc][1][:])
                    eng = nc.scalar if dc == 0 else nc.sync
                    eng.dma_start(g_out[:, dc * K:(dc + 1) * K],
                                  g_sb[:, dc * K:(dc + 1) * K])
                c_sb = accp.tile([1, K], F32, tag=f"csb_{it}")
                nc.scalar.activation(c_sb[:, 0:KC], c_ps[0][:], ACTF.Copy)
                nc.vector.tensor_copy(c_sb[:, KC:K], c_ps[1][:])
                cc_sb = accp.tile([128, 1], F32, tag=f"ccsb_{it}")
                nc.vector.tensor_reduce(cc_sb[:], rc[:],
                                        axis=mybir.AxisListType.X, op=ALU.add)
                nc.sync.dma_start(c_out, c_sb[:])
                nc.sync.dma_start(cc_out, cc_sb[:])

    if split_waits:
        _split_multi_waits(nc)
    return nc


def _build_lmap():
    lm = np.zeros((TOK, NG * TOK), dtype=BF)
    for g in range(NG):
        for p in range(TOK):
            lm[p, g * TOK + g * GRP + p // TNUM] = 1
    return lm


def pack_inputs(xs_pad_in, xs_pad_out, ilens, embed_w):
    ilens = np.asarray(ilens).astype(np.int64)
    ilens = np.minimum(np.maximum(ilens, 0), T)
    order = np.argsort(-ilens, kind="stable")
    assign = [[] for _ in range(NCORES)]
    loads = np.zeros(NCORES, dtype=np.int64)
    for bi in order:
        c = int(np.argmin(loads))
        assign[c].append(int(bi))
        loads[c] += int(ilens[bi])
    n_tiles = int(max(1, -(-int(loads.max()) // TOK)))
    n_tiles = -(-n_tiles // TPG) * TPG  # multiple of TPG (2) and of any tpg that divides it
    ntok = n_tiles * TOK

    inv = 1.0 / np.linalg.norm(np.asarray(embed_w, np.float32), axis=1)
    ets_np = (np.asarray(embed_w, np.float32) * inv[:, None]).T  # [D, K]
    ets_np = np.ascontiguousarray(ets_np).astype(BF)
    lmap_np = _build_lmap()

    xs_in = np.asarray(xs_pad_in, np.float32)
    xs_out = np.asarray(xs_pad_out, np.float32)

    in_maps = []
    for c in range(NCORES):
        nval = int(loads[c])
        xsb = np.zeros((ntok, TNUM, D), dtype=BF)
        xb = np.zeros((ntok, D), dtype=np.float32)
        pos = 0
        for bi in assign[c]:
            L = int(ilens[bi])
            if L > 0:
                xsb[pos:pos + L] = xs_out[bi, :L]
                xb[pos:pos + L] = xs_in[bi, :L]
                pos += L
        mk = np.zeros((128, n_tiles), dtype=np.float32)
        for j in range(n_tiles):
            lo = j * TOK
            n = min(max(nval - lo, 0), TOK)
            mk[:n, j] = float(MEAN)
        in_maps.append({
            "xs": xsb.reshape(ntok * TNUM, D),
            "xt": np.ascontiguousarray(xb.T).astype(BF),
            "ets": ets_np,
            "lmap": lmap_np,
            "mkf": mk,
        })
    return in_maps, n_tiles


_NC_CACHE = {}


def run_cores(in_maps, n_tiles, iters=1):
    key = (n_tiles, iters)
    if key not in _NC_CACHE:
        _NC_CACHE[key] = build_nc(n_tiles, iters)
    nc = _NC_CACHE[key]
    return run_bass_kernel_spmd(nc, in_maps, core_ids=list(range(NCORES)))


def combine(results, embed_w, W, b):
    E = np.asarray(embed_w, np.float64)
    Wf = np.asarray(W, np.float64)
    bf = np.asarray(b, np.float64)
    EWb = E @ Wf + bf                    # [K, D]
    s2 = (EWb * EWb).sum(axis=1)         # [K]
    sc = float(np.float32(MEAN).astype(BF))  # bf16(0.1) as used on device
    loss = 0.0
    for r in results:
        G = r["g_out"].astype(np.float64).reshape(128, 2, K)  # [p, dc, k]
        c = r["c_out"].astype(np.float64)[0] / sc             # counts
        C = r["cc_out"].astype(np.float64).sum()
        A = float(c @ s2)
        Bv = float((EWb[:, :128].T * G[:, 0, :]).sum()
                   + (EWb[:, 128:].T * G[:, 1, :]).sum())
        loss += 38.5 * A - 11.0 * Bv + C
    return np.asarray(loss, dtype=np.float32)


def kernel(xs_pad_in, xs_pad_out, ilens, ys_pad=None, embed_w=None, W=None, b=None):
    in_maps, n_tiles = pack_inputs(xs_pad_in, xs_pad_out, ilens, embed_w)
    res = run_cores(in_maps, n_tiles)
    return combine(res.results, embed_w, W, b)


# revision 29
# speedup vs baseline: 991.6037x; 1.0289x over previous
"""Trainium2 Bass kernel for nn_Net_44349832298833 (vq_codebook).

Math: the reference's 10-iteration loop collapses algebraically. With
  e   = embed_w[argmax_k (xs_pad_in @ (embed_w*inv_norm).T)] @ W + b
  t0  = mean(xs_pad_out, axis=-2)
  v   = valid mask from ilens
  A   = sum(v*|e|^2),  B = sum(v*e.t0),  C = sum(v*|t0|^2)
the loss is  38.5*A - 11*B + C   (sum_{i=1..10} ((i*e - t0)^2 * v) / 10).

Device work per token: score matmul (K=1000) + argmax -> onehot,
t0 via block-ones matmul over the 10 xs_pad_out slices, then
G[d,k] = sum_t v*t0[t,d]*onehot[t,k] and c[k] = sum_t v*onehot[t,k]
accumulated in PSUM across all tiles. Host folds weights (EWb = E@W+b)
and combines: A = (c/s)@|EWb|^2-rows, B = sum(EWb.T * G), C from device.

Sharding: data-parallel over B across 8 cores, batches greedily balanced
by sum(ilens); only the valid token prefixes are shipped (packed dense),
in bf16. Same program on all cores (token count padded to a common tile
count); padding tokens carry a zero mask so they contribute nothing.
"""
import os
import sys
import numpy as np

for _p in ("/opt/trn_rl_repo", "/root/.axon_site/_ro/trn_rl_repo"):
    if os.path.isdir(_p) and _p not in sys.path:
        sys.path.insert(0, _p)

import ml_dtypes
import concourse.bass as bass
import concourse.tile as tile
from concourse import mybir
from concourse.bass_utils import run_bass_kernel_spmd

F32 = mybir.dt.float32
BF16 = mybir.dt.bfloat16
BF = ml_dtypes.bfloat16
ACTF = mybir.ActivationFunctionType
ALU = mybir.AluOpType

B, T, D, TNUM, K = 32, 512, 256, 10, 1000
NCORES = 8
TOK = 120          # tokens per tile
GRP = 12           # tokens per t0-matmul group (12*10 = 120 partitions)
NG = TOK // GRP    # 10 groups per tile
TPG = 2            # tiles per supertile (DMA batching)
KC = 500           # score/G k-chunk (psum bank = 500 f32)
MEAN = np.float32(1.0 / TNUM)


def _split_multi_waits(nc, max_waits=1):
    # This container's walrus supports one sync-wait per instruction;
    # hoist extras into standalone NoOps on the same engine.
    n = 0
    for fn in nc.m.functions:
        for bb in fn.blocks:
            out = []
            for inst in bb.instructions:
                si = inst.sync_info
                if si is not None and si.on_wait and len(si.on_wait) > max_waits:
                    for w in list(si.on_wait)[:-max_waits]:
                        out.append(mybir.InstNoOp(
                            name=f"waitsplit-{nc.next_id()}",
                            sync_info=mybir.SyncInfo(on_wait=[w], on_update=[]),
                            bass_nofuse=True,
                            engine=inst.engine,
                        ))
                        n += 1
                    si.on_wait = list(si.on_wait)[-max_waits:]
                out.append(inst)
            bb.instructions = out
    return n


def build_nc(n_tiles: int, iters: int = 1, split_waits: bool = True,
             tpg: int = TPG, xs_bufs: int = 2, work_bufs: int = 3,
             dve_t0_mod: int = 5, dve_t0_cnt: int = 4, pool_t0_cnt: int = 2):
    """One SPMD program for all 8 cores, n_tiles tiles of TOK tokens each.
    iters>1 repeats the whole computation (for wall-clock benchmarking)."""
    assert n_tiles % tpg == 0
    nst = n_tiles // tpg
    ntok = n_tiles * TOK

    nc = bass.Bass("TRN2", target_bir_lowering=False, debug=False)
    xs = nc.dram_tensor("xs", [ntok * TNUM, D], BF16, kind="ExternalInput").ap()
    xt = nc.dram_tensor("xt", [D, ntok], BF16, kind="ExternalInput").ap()
    ets = nc.dram_tensor("ets", [D, K], BF16, kind="ExternalInput").ap()
    lmap = nc.dram_tensor("lmap", [TOK, NG * TOK], BF16, kind="ExternalInput").ap()
    mkf = nc.dram_tensor("mkf", [128, n_tiles], F32, kind="ExternalInput").ap()
    g_out = nc.dram_tensor("g_out", [128, 2 * K], BF16, kind="ExternalOutput").ap()
    c_out = nc.dram_tensor("c_out", [1, K], F32, kind="ExternalOutput").ap()
    cc_out = nc.dram_tensor("cc_out", [128, 1], F32, kind="ExternalOutput").ap()

    # DRAM views. xs_v: (token,i)-interleaved partitions for the block-ones
    # matmul; xs_tok: token-major partitions (i in free dim) for the DVE
    # pairwise-tree tiles — same bytes, different access pattern.
    xs_v = xs.rearrange("(st jj g p) d -> st p jj g d", st=nst, jj=tpg, g=NG, p=TOK)
    xs_tok = xs.rearrange("(st jj p i) d -> st p jj i d", st=nst, jj=tpg, p=TOK, i=TNUM)
    xt_v = xt.rearrange("(dc p) (st tt) -> st p dc tt", dc=2, p=128, st=nst)
    ets_v = ets.rearrange("(dc p) k -> p dc k", dc=2)

    with tile.TileContext(nc) as tc:
        with tc.tile_pool(name="const", bufs=1) as constp, \
             tc.tile_pool(name="xsp", bufs=xs_bufs) as xsp, \
             tc.tile_pool(name="xtp", bufs=2) as xtp, \
             tc.tile_pool(name="work", bufs=work_bufs) as work, \
             tc.tile_pool(name="acc", bufs=1) as accp, \
             tc.tile_pool(name="gps", bufs=1, space="PSUM") as gps, \
             tc.tile_pool(name="cps", bufs=1, space="PSUM") as cps, \
             tc.tile_pool(name="tps", bufs=2, space="PSUM") as tps:

            # warm the ACT function tables while DMAs run
            warm = constp.tile([1, 1], F32, tag="warm")
            nc.vector.memset(warm[:], 0.0)
            warm2 = constp.tile([1, 1], BF16, tag="warm2")
            nc.scalar.activation(warm2[:], warm[:], ACTF.Copy)

            # constants on the ACT HWDGE ring; xs stream owns the SP ring
            ets_sb = constp.tile([128, 2, K], BF16, tag="ets")
            lmap_sb = constp.tile([TOK, NG * TOK], BF16, tag="lmap")
            mkf_sb = constp.tile([128, n_tiles], F32, tag="mkf")
            mkh_sb = constp.tile([128, n_tiles], BF16, tag="mkh")
            nc.scalar.dma_start(ets_sb[:], ets_v)
            nc.scalar.dma_start(lmap_sb[:], lmap)
            nc.scalar.dma_start(mkf_sb[:], mkf)
            nc.vector.tensor_copy(mkh_sb[:], mkf_sb[:])  # bf16(0.1*v)

            for it in range(iters):
                g_ps = [[gps.tile([128, KC], F32, tag=f"g{dc}{kc}", name=f"g{dc}{kc}")
                         for kc in range(2)] for dc in range(2)]
                c_ps = [cps.tile([1, KC], F32, tag=f"c{kc}", name=f"c{kc}") for kc in range(2)]
                rc = accp.tile([128, n_tiles], F32, tag="rc")
                nc.vector.memset(rc[:], 0.0)

                for st in range(nst):
                    xs_sb = xsp.tile([TOK, tpg, NG, D], BF16, tag="xs")
                    xt_sb = xtp.tile([128, 2, tpg * TOK], BF16, tag="xt")
                    # per-tile loads: tree tiles get the token-major view,
                    # matmul tiles the (token,i)-interleaved view
                    for jj in range(tpg):
                        j = st * tpg + jj
                        view = (xs_tok if j % dve_t0_mod < dve_t0_cnt
                                else xs_v)
                        nc.sync.dma_start(xs_sb[:, jj:jj + 1],
                                          view[st][:, jj:jj + 1])
                        if st == 0 and it == 0 and jj == 0:
                            nc.sync.dma_start(xt_sb[:], xt_v[st])
                    if not (st == 0 and it == 0):
                        nc.scalar.dma_start(xt_sb[:], xt_v[st])

                    for jj in range(tpg):
                        j = st * tpg + jj
                        first = j == 0
                        last = j == n_tiles - 1

                        # score chunks -> psum, drained to SBUF by ACT fast
                        # so the transient psum slots never wait on DVE
                        s_ps = [tps.tile([TOK, KC], F32, tag="t", name=f"s{i}") for i in range(2)]
                        for kc in range(2):
                            for dc in range(2):
                                nc.tensor.matmul(
                                    s_ps[kc][:],
                                    xt_sb[:, dc, jj * TOK:(jj + 1) * TOK],
                                    ets_sb[:, dc, kc * KC:(kc + 1) * KC],
                                    start=(dc == 0), stop=(dc == 1))
                        sc_sb = work.tile([TOK, 2 * KC], F32, tag="sc")
                        for kc in range(2):
                            nc.scalar.activation(sc_sb[:, kc * KC:(kc + 1) * KC],
                                                 s_ps[kc][:], ACTF.Copy)

                        # t0 = sum_i xs[t, i, :]. PE (block-ones matmuls) is
                        # the busiest engine, so a slice of tiles computes it
                        # on DVE instead (bf16 pairwise tree runs at 2x mode)
                        # to balance engine load.
                        t0v = work.tile([TOK, D], BF16, tag="t0v")
                        if j % dve_t0_mod < dve_t0_cnt:
                            # a slice of tree tiles runs on the idle GpSimd
                            eng = (nc.gpsimd if j % dve_t0_mod < pool_t0_cnt
                                   else nc.vector)
                            tA = work.tile([TOK, 5, D], BF16, tag="tA")
                            eng.tensor_add(tA[:], xs_sb[:, jj, 0:5, :],
                                           xs_sb[:, jj, 5:10, :])
                            tB = work.tile([TOK, 2, D], BF16, tag="tB")
                            eng.tensor_add(tB[:], tA[:, 0:2, :],
                                           tA[:, 2:4, :])
                            tC = work.tile([TOK, D], BF16, tag="tC")
                            eng.tensor_add(tC[:], tB[:, 0, :], tB[:, 1, :])
                            tD = work.tile([TOK, D], BF16, tag="tD")
                            eng.tensor_add(tD[:], tC[:], tA[:, 4, :])
                            nc.scalar.activation(t0v[:], tD[:], ACTF.Copy,
                                                 scale=mkf_sb[0:TOK, j:j + 1])
                        else:
                            t0_ps = tps.tile([TOK, D], F32, tag="t")
                            for g in range(NG):
                                nc.tensor.matmul(
                                    t0_ps[:],
                                    lmap_sb[:, g * TOK:(g + 1) * TOK],
                                    xs_sb[:, jj, g, :],
                                    start=(g == 0), stop=(g == NG - 1))
                            # t0v = (0.1 * v) * t0  (psum -> sbuf bf16)
                            nc.scalar.activation(t0v[:], t0_ps[:], ACTF.Copy,
                                                 scale=mkf_sb[0:TOK, j:j + 1])

                        # row max and onehot = (score == max) -> bf16, from SBUF
                        mm = work.tile([TOK, 1], F32, tag="mm")
                        nc.vector.tensor_reduce(mm[:], sc_sb[:],
                                                axis=mybir.AxisListType.X, op=ALU.max)
                        oh = work.tile([TOK, 2 * KC], BF16, tag="oh")
                        nc.vector.tensor_scalar(oh[:], sc_sb[:],
                                                mm[:, 0:1], None, op0=ALU.is_equal)

                        # G[dc][kc] += t0v.T-chunk x onehot ; c[kc] += v x onehot
                        for kc in range(2):
                            for dc in range(2):
                                nc.tensor.matmul(
                                    g_ps[dc][kc][:],
                                    t0v[:, dc * 128:(dc + 1) * 128],
                                    oh[:, kc * KC:(kc + 1) * KC],
                                    start=first, stop=last)
                            nc.tensor.matmul(
                                c_ps[kc][:],
                                mkh_sb[0:TOK, j:j + 1],
                                oh[:, kc * KC:(kc + 1) * KC],
                                start=first, stop=last)

                        # C: accumulate |t0v|^2 per partition into rc[:, j]
                        sq = work.tile([TOK, D], BF16, tag="sq")
                        nc.vector.scalar_tensor_tensor(
                            sq[:], t0v[:], 1.0, t0v[:],
                            op0=ALU.bypass, op1=ALU.mult,
                            accum_out=rc[0:TOK, j:j + 1])

                # outputs (overwritten every iter; identical data).
                # psum drains split across ACT and DVE; G shipped as bf16.
                g_sb = accp.tile([128, 2 * K], BF16, tag="gsb")
                for dc in range(2):
                    dst = g_sb[:, dc * K: dc * K + KC]
                    dst2 = g_sb[:, dc * K + KC: (dc + 1) * K]
                    nc.scalar.activation(dst, g_ps[dc][0][:], ACTF.Copy)
                    nc.vector.tensor_copy(dst2, g_ps[dc][1][:])
                    eng = nc.scalar if dc == 0 else nc.sync
                    eng.dma_start(g_out[:, dc * K:(dc + 1) * K],
                                  g_sb[:, dc * K:(dc + 1) * K])
                c_sb = accp.tile([1, K], F32, tag=f"csb_{it}")
                nc.scalar.activation(c_sb[:, 0:KC], c_ps[0][:], ACTF.Copy)
                nc.vector.tensor_copy(c_sb[:, KC:K], c_ps[1][:])
                cc_sb = accp.tile([128, 1], F32, tag=f"ccsb_{it}")
                nc.vector.tensor_reduce(cc_sb[:], rc[:],
                                        axis=mybir.AxisListType.X, op=ALU.add)
                nc.sync.dma_start(c_out, c_sb[:])
                nc.sync.dma_start(cc_out, cc_sb[:])

    if split_waits:
        _split_multi_waits(nc)
    return nc


def _build_lmap():
    lm = np.zeros((TOK, NG * TOK), dtype=BF)
    for g in range(NG):
        for p in range(TOK):
            lm[p, g * TOK + g * GRP + p // TNUM] = 1
    return lm


def pack_inputs(xs_pad_in, xs_pad_out, ilens, embed_w):
    ilens = np.asarray(ilens).astype(np.int64)
    ilens = np.minimum(np.maximum(ilens, 0), T)
    order = np.argsort(-ilens, kind="stable")
    assign = [[] for _ in range(NCORES)]
    loads = np.zeros(NCORES, dtype=np.int64)
    for bi in order:
        c = int(np.argmin(loads))
        assign[c].append(int(bi))
        loads[c] += int(ilens[bi])
    n_tiles = int(max(1, -(-int(loads.max()) // TOK)))
    n_tiles = -(-n_tiles // TPG) * TPG  # multiple of TPG (2) and of any tpg that divides it
    ntok = n_tiles * TOK

    inv = 1.0 / np.linalg.norm(np.asarray(embed_w, np.float32), axis=1)
    ets_np = (np.asarray(embed_w, np.float32) * inv[:, None]).T  # [D, K]
    ets_np = np.ascontiguousarray(ets_np).astype(BF)
    lmap_np = _build_lmap()

    xs_in = np.asarray(xs_pad_in, np.float32)
    xs_out = np.asarray(xs_pad_out, np.float32)

    in_maps = []
    for c in range(NCORES):
        nval = int(loads[c])
        xsb = np.zeros((ntok, TNUM, D), dtype=BF)
        xb = np.zeros((ntok, D), dtype=np.float32)
        pos = 0
        for bi in assign[c]:
            L = int(ilens[bi])
            if L > 0:
                xsb[pos:pos + L] = xs_out[bi, :L]
                xb[pos:pos + L] = xs_in[bi, :L]
                pos += L
        mk = np.zeros((128, n_tiles), dtype=np.float32)
        for j in range(n_tiles):
            lo = j * TOK
            n = min(max(nval - lo, 0), TOK)
            mk[:n, j] = float(MEAN)
        in_maps.append({
            "xs": xsb.reshape(ntok * TNUM, D),
            "xt": np.ascontiguousarray(xb.T).astype(BF),
            "ets": ets_np,
            "lmap": lmap_np,
            "mkf": mk,
        })
    return in_maps, n_tiles


_NC_CACHE = {}


def run_cores(in_maps, n_tiles, iters=1):
    key = (n_tiles, iters)
    if key not in _NC_CACHE:
        _NC_CACHE[key] = build_nc(n_tiles, iters)
    nc = _NC_CACHE[key]
    return run_bass_kernel_spmd(nc, in_maps, core_ids=list(range(NCORES)))


def combine(results, embed_w, W, b):
    E = np.asarray(embed_w, np.float64)
    Wf = np.asarray(W, np.float64)
    bf = np.asarray(b, np.float64)
    EWb = E @ Wf + bf                    # [K, D]
    s2 = (EWb * EWb).sum(axis=1)         # [K]
    sc = float(np.float32(MEAN).astype(BF))  # bf16(0.1) as used on device
    loss = 0.0
    for r in results:
        G = r["g_out"].astype(np.float64).reshape(128, 2, K)  # [p, dc, k]
        c = r["c_out"].astype(np.float64)[0] / sc             # counts
        C = r["cc_out"].astype(np.float64).sum()
        A = float(c @ s2)
        Bv = float((EWb[:, :128].T * G[:, 0, :]).sum()
                   + (EWb[:, 128:].T * G[:, 1, :]).sum())
        loss += 38.5 * A - 11.0 * Bv + C
    return np.asarray(loss, dtype=np.float32)


def kernel(xs_pad_in, xs_pad_out, ilens, ys_pad=None, embed_w=None, W=None, b=None):
    in_maps, n_tiles = pack_inputs(xs_pad_in, xs_pad_out, ilens, embed_w)
    res = run_cores(in_maps, n_tiles)
    return combine(res.results, embed_w, W, b)


# revision 32
# speedup vs baseline: 1111.4625x; 1.1209x over previous
"""Trainium2 Bass kernel for nn_Net_44349832298833 (vq_codebook).

Math: the reference's 10-iteration loop collapses algebraically. With
  e   = embed_w[argmax_k (xs_pad_in @ (embed_w*inv_norm).T)] @ W + b
  t0  = mean(xs_pad_out, axis=-2)
  v   = valid mask from ilens
  A   = sum(v*|e|^2),  B = sum(v*e.t0),  C = sum(v*|t0|^2)
the loss is  38.5*A - 11*B + C   (sum_{i=1..10} ((i*e - t0)^2 * v) / 10).

Device work per token: score matmul (K=1000) + argmax -> onehot,
t0 via block-ones matmul over the 10 xs_pad_out slices, then
G[d,k] = sum_t v*t0[t,d]*onehot[t,k] and c[k] = sum_t v*onehot[t,k]
accumulated in PSUM across all tiles. Host folds weights (EWb = E@W+b)
and combines: A = (c/s)@|EWb|^2-rows, B = sum(EWb.T * G), C from device.

Sharding: data-parallel over B across 8 cores, batches greedily balanced
by sum(ilens); only the valid token prefixes are shipped (packed dense),
in bf16. Same program on all cores (token count padded to a common tile
count); padding tokens carry a zero mask so they contribute nothing.
"""
import os
import sys
import numpy as np

for _p in ("/opt/trn_rl_repo", "/root/.axon_site/_ro/trn_rl_repo"):
    if os.path.isdir(_p) and _p not in sys.path:
        sys.path.insert(0, _p)

import ml_dtypes
import concourse.bass as bass
import concourse.tile as tile
from concourse import mybir
from concourse.bass_utils import run_bass_kernel_spmd

F32 = mybir.dt.float32
BF16 = mybir.dt.bfloat16
BF = ml_dtypes.bfloat16
ACTF = mybir.ActivationFunctionType
ALU = mybir.AluOpType

B, T, D, TNUM, K = 32, 512, 256, 10, 1000
NCORES = 8
TOK = 120          # tokens per tile
GRP = 12           # tokens per t0-matmul group (12*10 = 120 partitions)
NG = TOK // GRP    # 10 groups per tile
TPG = 2            # tiles per supertile (DMA batching)
KC = 500           # score/G k-chunk (psum bank = 500 f32)
MEAN = np.float32(1.0 / TNUM)


def _split_multi_waits(nc, max_waits=1):
    # This container's walrus supports one sync-wait per instruction;
    # hoist extras into standalone NoOps on the same engine.
    n = 0
    for fn in nc.m.functions:
        for bb in fn.blocks:
            out = []
            for inst in bb.instructions:
                si = inst.sync_info
                if si is not None and si.on_wait and len(si.on_wait) > max_waits:
                    for w in list(si.on_wait)[:-max_waits]:
                        out.append(mybir.InstNoOp(
                            name=f"waitsplit-{nc.next_id()}",
                            sync_info=mybir.SyncInfo(on_wait=[w], on_update=[]),
                            bass_nofuse=True,
                            engine=inst.engine,
                        ))
                        n += 1
                    si.on_wait = list(si.on_wait)[-max_waits:]
                out.append(inst)
            bb.instructions = out
    return n


def build_nc(n_tiles: int, iters: int = 1, split_waits: bool = True,
             tpg: int = TPG, xs_bufs: int = 2, work_bufs: int = 3,
             dve_t0_mod: int = 5, dve_t0_cnt: int = 5, pool_t0_cnt: int = 5):
    """One SPMD program for all 8 cores, n_tiles tiles of TOK tokens each.
    iters>1 repeats the whole computation (for wall-clock benchmarking)."""
    assert n_tiles % tpg == 0
    nst = n_tiles // tpg
    ntok = n_tiles * TOK

    nc = bass.Bass("TRN2", target_bir_lowering=False, debug=False)
    xs = nc.dram_tensor("xs", [ntok * TNUM, D], BF16, kind="ExternalInput").ap()
    xt = nc.dram_tensor("xt", [D, ntok], BF16, kind="ExternalInput").ap()
    ets = nc.dram_tensor("ets", [D, K], BF16, kind="ExternalInput").ap()
    lmap = nc.dram_tensor("lmap", [TOK, NG * TOK], BF16, kind="ExternalInput").ap()
    mkf = nc.dram_tensor("mkf", [128, n_tiles], F32, kind="ExternalInput").ap()
    g_out = nc.dram_tensor("g_out", [128, 2 * K], BF16, kind="ExternalOutput").ap()
    c_out = nc.dram_tensor("c_out", [1, K], F32, kind="ExternalOutput").ap()
    cc_out = nc.dram_tensor("cc_out", [128, 1], F32, kind="ExternalOutput").ap()

    # DRAM views. xs_v: (token,i)-interleaved partitions for the block-ones
    # matmul; xs_tok: token-major partitions (i in free dim) for the DVE
    # pairwise-tree tiles — same bytes, different access pattern.
    xs_v = xs.rearrange("(st jj g p) d -> st p jj g d", st=nst, jj=tpg, g=NG, p=TOK)
    xs_tok = xs.rearrange("(st jj p i) d -> st p jj i d", st=nst, jj=tpg, p=TOK, i=TNUM)
    xt_v = xt.rearrange("(dc p) (st tt) -> st p dc tt", dc=2, p=128, st=nst)
    ets_v = ets.rearrange("(dc p) k -> p dc k", dc=2)

    with tile.TileContext(nc) as tc:
        with tc.tile_pool(name="const", bufs=1) as constp, \
             tc.tile_pool(name="xsp", bufs=xs_bufs) as xsp, \
             tc.tile_pool(name="xtp", bufs=2) as xtp, \
             tc.tile_pool(name="work", bufs=work_bufs) as work, \
             tc.tile_pool(name="acc", bufs=1) as accp, \
             tc.tile_pool(name="gps", bufs=1, space="PSUM") as gps, \
             tc.tile_pool(name="cps", bufs=1, space="PSUM") as cps, \
             tc.tile_pool(name="tps", bufs=2, space="PSUM") as tps:

            # warm the ACT function tables while DMAs run
            warm = constp.tile([1, 1], F32, tag="warm")
            nc.vector.memset(warm[:], 0.0)
            warm2 = constp.tile([1, 1], BF16, tag="warm2")
            nc.scalar.activation(warm2[:], warm[:], ACTF.Copy)

            # constants on the ACT HWDGE ring; xs stream owns the SP ring
            ets_sb = constp.tile([128, 2, K], BF16, tag="ets")
            lmap_sb = constp.tile([TOK, NG * TOK], BF16, tag="lmap")
            mkf_sb = constp.tile([128, n_tiles], F32, tag="mkf")
            mkh_sb = constp.tile([128, n_tiles], BF16, tag="mkh")
            nc.scalar.dma_start(ets_sb[:], ets_v)
            if dve_t0_cnt < dve_t0_mod:  # lmap only used by PE-t0 tiles
                nc.scalar.dma_start(lmap_sb[:], lmap)
            nc.scalar.dma_start(mkf_sb[:], mkf)
            nc.vector.tensor_copy(mkh_sb[:], mkf_sb[:])  # bf16(0.1*v)

            for it in range(iters):
                g_ps = [[gps.tile([128, KC], F32, tag=f"g{dc}{kc}", name=f"g{dc}{kc}")
                         for kc in range(2)] for dc in range(2)]
                c_ps = [cps.tile([1, KC], F32, tag=f"c{kc}", name=f"c{kc}") for kc in range(2)]
                rc = accp.tile([128, n_tiles], F32, tag="rc")
                nc.vector.memset(rc[:], 0.0)

                for st in range(nst):
                    xs_sb = xsp.tile([TOK, tpg, NG, D], BF16, tag="xs")
                    xt_sb = xtp.tile([128, 2, tpg * TOK], BF16, tag="xt")
                    # per-tile loads: tree tiles get the token-major view,
                    # matmul tiles the (token,i)-interleaved view
                    for jj in range(tpg):
                        j = st * tpg + jj
                        view = (xs_tok if j % dve_t0_mod < dve_t0_cnt
                                else xs_v)
                        nc.sync.dma_start(xs_sb[:, jj:jj + 1],
                                          view[st][:, jj:jj + 1])
                        if st == 0 and it == 0 and jj == 0:
                            nc.sync.dma_start(xt_sb[:], xt_v[st])
                    if not (st == 0 and it == 0):
                        nc.scalar.dma_start(xt_sb[:], xt_v[st])

                    for jj in range(tpg):
                        j = st * tpg + jj
                        first = j == 0
                        last = j == n_tiles - 1

                        # score chunks -> psum, drained to SBUF by ACT fast
                        # so the transient psum slots never wait on DVE
                        s_ps = [tps.tile([TOK, KC], F32, tag="t", name=f"s{i}") for i in range(2)]
                        for kc in range(2):
                            for dc in range(2):
                                nc.tensor.matmul(
                                    s_ps[kc][:],
                                    xt_sb[:, dc, jj * TOK:(jj + 1) * TOK],
                                    ets_sb[:, dc, kc * KC:(kc + 1) * KC],
                                    start=(dc == 0), stop=(dc == 1))
                        sc_sb = work.tile([TOK, 2 * KC], F32, tag="sc")
                        for kc in range(2):
                            nc.scalar.activation(sc_sb[:, kc * KC:(kc + 1) * KC],
                                                 s_ps[kc][:], ACTF.Copy)

                        # t0 = sum_i xs[t, i, :]. PE (block-ones matmuls) is
                        # the busiest engine, so a slice of tiles computes it
                        # on DVE instead (bf16 pairwise tree runs at 2x mode)
                        # to balance engine load.
                        t0v = work.tile([TOK, D], BF16, tag="t0v")
                        if j % dve_t0_mod < dve_t0_cnt:
                            # a slice of tree tiles runs on the idle GpSimd
                            eng = (nc.gpsimd if j % dve_t0_mod < pool_t0_cnt
                                   else nc.vector)
                            tA = work.tile([TOK, 5, D], BF16, tag="tA")
                            eng.tensor_add(tA[:], xs_sb[:, jj, 0:5, :],
                                           xs_sb[:, jj, 5:10, :])
                            tB = work.tile([TOK, 2, D], BF16, tag="tB")
                            eng.tensor_add(tB[:], tA[:, 0:2, :],
                                           tA[:, 2:4, :])
                            tC = work.tile([TOK, D], BF16, tag="tC")
                            eng.tensor_add(tC[:], tB[:, 0, :], tB[:, 1, :])
                            tD = work.tile([TOK, D], BF16, tag="tD")
                            eng.tensor_add(tD[:], tC[:], tA[:, 4, :])
                            nc.vector.tensor_scalar(
                                t0v[:], tD[:], mkf_sb[0:TOK, j:j + 1], None,
                                op0=ALU.mult)
                        else:
                            t0_ps = tps.tile([TOK, D], F32, tag="t")
                            for g in range(NG):
                                nc.tensor.matmul(
                                    t0_ps[:],
                                    lmap_sb[:, g * TOK:(g + 1) * TOK],
                                    xs_sb[:, jj, g, :],
                                    start=(g == 0), stop=(g == NG - 1))
                            # t0v = (0.1 * v) * t0  (psum -> sbuf bf16)
                            nc.scalar.activation(t0v[:], t0_ps[:], ACTF.Copy,
                                                 scale=mkf_sb[0:TOK, j:j + 1])

                        # row max and onehot = (score == max) -> bf16, from SBUF
                        mm = work.tile([TOK, 1], F32, tag="mm")
                        nc.vector.tensor_reduce(mm[:], sc_sb[:],
                                                axis=mybir.AxisListType.X, op=ALU.max)
                        oh = work.tile([TOK, 2 * KC], BF16, tag="oh")
                        nc.vector.tensor_scalar(oh[:], sc_sb[:],
                                                mm[:, 0:1], None, op0=ALU.is_equal)

                        # G[dc][kc] += t0v.T-chunk x onehot ; c[kc] += v x onehot
                        for kc in range(2):
                            for dc in range(2):
                                nc.tensor.matmul(
                                    g_ps[dc][kc][:],
                                    t0v[:, dc * 128:(dc + 1) * 128],
                                    oh[:, kc * KC:(kc + 1) * KC],
                                    start=first, stop=last)
                            nc.tensor.matmul(
                                c_ps[kc][:],
                                mkh_sb[0:TOK, j:j + 1],
                                oh[:, kc * KC:(kc + 1) * KC],
                                start=first, stop=last)

                        # C: accumulate |t0v|^2 per partition into rc[:, j]
                        # (tree tiles put it on the Pool engine)
                        sq = work.tile([TOK, D], BF16, tag="sq")
                        nc.vector.scalar_tensor_tensor(
                            sq[:], t0v[:], 1.0, t0v[:],
                            op0=ALU.bypass, op1=ALU.mult,
                            accum_out=rc[0:TOK, j:j + 1])

                # outputs (overwritten every iter; identical data).
                # psum drains split across ACT and DVE; G shipped as bf16.
                g_sb = accp.tile([128, 2 * K], BF16, tag="gsb")
                for dc in range(2):
                    dst = g_sb[:, dc * K: dc * K + KC]
                    dst2 = g_sb[:, dc * K + KC: (dc + 1) * K]
                    nc.scalar.activation(dst, g_ps[dc][0][:], ACTF.Copy)
                    nc.vector.tensor_copy(dst2, g_ps[dc][1][:])
                    eng = nc.scalar if dc == 0 else nc.sync
                    eng.dma_start(g_out[:, dc * K:(dc + 1) * K],
                                  g_sb[:, dc * K:(dc + 1) * K])
                c_sb = accp.tile([1, K], F32, tag=f"csb_{it}")
                nc.scalar.activation(c_sb[:, 0:KC], c_ps[0][:], ACTF.Copy)
                nc.vector.tensor_copy(c_sb[:, KC:K], c_ps[1][:])
                cc_sb = accp.tile([128, 1], F32, tag=f"ccsb_{it}")
                nc.vector.tensor_reduce(cc_sb[:], rc[:],
                                        axis=mybir.AxisListType.X, op=ALU.add)
                nc.sync.dma_start(c_out, c_sb[:])
                nc.sync.dma_start(cc_out, cc_sb[:])

    if split_waits:
        _split_multi_waits(nc)
    return nc


def _build_lmap():
    lm = np.zeros((TOK, NG * TOK), dtype=BF)
    for g in range(NG):
        for p in range(TOK):
            lm[p, g * TOK + g * GRP + p // TNUM] = 1
    return lm


def pack_inputs(xs_pad_in, xs_pad_out, ilens, embed_w):
    ilens = np.asarray(ilens).astype(np.int64)
    ilens = np.minimum(np.maximum(ilens, 0), T)
    order = np.argsort(-ilens, kind="stable")
    assign = [[] for _ in range(NCORES)]
    loads = np.zeros(NCORES, dtype=np.int64)
    for bi in order:
        c = int(np.argmin(loads))
        assign[c].append(int(bi))
        loads[c] += int(ilens[bi])
    n_tiles = int(max(1, -(-int(loads.max()) // TOK)))
    n_tiles = -(-n_tiles // TPG) * TPG  # multiple of TPG (2) and of any tpg that divides it
    ntok = n_tiles * TOK

    inv = 1.0 / np.linalg.norm(np.asarray(embed_w, np.float32), axis=1)
    ets_np = (np.asarray(embed_w, np.float32) * inv[:, None]).T  # [D, K]
    ets_np = np.ascontiguousarray(ets_np).astype(BF)
    lmap_np = _build_lmap()

    xs_in = np.asarray(xs_pad_in, np.float32)
    xs_out = np.asarray(xs_pad_out, np.float32)

    in_maps = []
    for c in range(NCORES):
        nval = int(loads[c])
        xsb = np.zeros((ntok, TNUM, D), dtype=BF)
        xb = np.zeros((ntok, D), dtype=np.float32)
        pos = 0
        for bi in assign[c]:
            L = int(ilens[bi])
            if L > 0:
                xsb[pos:pos + L] = xs_out[bi, :L]
                xb[pos:pos + L] = xs_in[bi, :L]
                pos += L
        mk = np.zeros((128, n_tiles), dtype=np.float32)
        for j in range(n_tiles):
            lo = j * TOK
            n = min(max(nval - lo, 0), TOK)
            mk[:n, j] = float(MEAN)
        in_maps.append({
            "xs": xsb.reshape(ntok * TNUM, D),
            "xt": np.ascontiguousarray(xb.T).astype(BF),
            "ets": ets_np,
            "lmap": lmap_np,
            "mkf": mk,
        })
    return in_maps, n_tiles


_NC_CACHE = {}


def run_cores(in_maps, n_tiles, iters=1):
    key = (n_tiles, iters)
    if key not in _NC_CACHE:
        _NC_CACHE[key] = build_nc(n_tiles, iters)
    nc = _NC_CACHE[key]
    return run_bass_kernel_spmd(nc, in_maps, core_ids=list(range(NCORES)))


def combine(results, embed_w, W, b):
    E = np.asarray(embed_w, np.float64)
    Wf = np.asarray(W, np.float64)
    bf = np.asarray(b, np.float64)
    EWb = E @ Wf + bf                    # [K, D]
    s2 = (EWb * EWb).sum(axis=1)         # [K]
    sc = float(np.float32(MEAN).astype(BF))  # bf16(0.1) as used on device
    loss = 0.0
    for r in results:
        G = r["g_out"].astype(np.float64).reshape(128, 2, K)  # [p, dc, k]
        c = r["c_out"].astype(np.float64)[0] / sc             # counts
        C = r["cc_out"].astype(np.float64).sum()
        A = float(c @ s2)
        Bv = float((EWb[:, :128].T * G[:, 0, :]).sum()
                   + (EWb[:, 128:].T * G[:, 1, :]).sum())
        loss += 38.5 * A - 11.0 * Bv + C
    return np.asarray(loss, dtype=np.float32)


def kernel(xs_pad_in, xs_pad_out, ilens, ys_pad=None, embed_w=None, W=None, b=None):
    in_maps, n_tiles = pack_inputs(xs_pad_in, xs_pad_out, ilens, embed_w)
    res = run_cores(in_maps, n_tiles)
    return combine(res.results, embed_w, W, b)


# revision 34
# speedup vs baseline: 1192.5339x; 1.0729x over previous
"""Trainium2 Bass kernel for nn_Net_44349832298833 (vq_codebook).

Math: the reference's 10-iteration loop collapses algebraically. With
  e   = embed_w[argmax_k (xs_pad_in @ (embed_w*inv_norm).T)] @ W + b
  t0  = mean(xs_pad_out, axis=-2)
  v   = valid mask from ilens
  A   = sum(v*|e|^2),  B = sum(v*e.t0),  C = sum(v*|t0|^2)
the loss is  38.5*A - 11*B + C   (sum_{i=1..10} ((i*e - t0)^2 * v) / 10).

Device work per token: score matmul (K=1000) + argmax -> onehot,
t0 via block-ones matmul over the 10 xs_pad_out slices, then
G[d,k] = sum_t v*t0[t,d]*onehot[t,k] and c[k] = sum_t v*onehot[t,k]
accumulated in PSUM across all tiles. Host folds weights (EWb = E@W+b)
and combines: A = (c/s)@|EWb|^2-rows, B = sum(EWb.T * G), C from device.

Sharding: data-parallel over B across 8 cores, batches greedily balanced
by sum(ilens); only the valid token prefixes are shipped (packed dense),
in bf16. Same program on all cores (token count padded to a common tile
count); padding tokens carry a zero mask so they contribute nothing.
"""
import os
import sys
import numpy as np

for _p in ("/opt/trn_rl_repo", "/root/.axon_site/_ro/trn_rl_repo"):
    if os.path.isdir(_p) and _p not in sys.path:
        sys.path.insert(0, _p)

import ml_dtypes
import concourse.bass as bass
import concourse.tile as tile
from concourse import mybir
from concourse.bass_utils import run_bass_kernel_spmd

F32 = mybir.dt.float32
BF16 = mybir.dt.bfloat16
BF = ml_dtypes.bfloat16
ACTF = mybir.ActivationFunctionType
ALU = mybir.AluOpType

B, T, D, TNUM, K = 32, 512, 256, 10, 1000
NCORES = 8
TOK = 128          # tokens per tile (tree path needs no partition interleave)
GRP = 12           # tokens per t0-matmul group (12*10 = 120 partitions)
NG = TOK // GRP    # 10 groups per tile
TPG = 3            # tiles per supertile (DMA batching)
KC = 500           # score/G k-chunk (psum bank = 500 f32)
MEAN = np.float32(1.0 / TNUM)


def _split_multi_waits(nc, max_waits=1):
    # This container's walrus supports one sync-wait per instruction;
    # hoist extras into standalone NoOps on the same engine.
    n = 0
    for fn in nc.m.functions:
        for bb in fn.blocks:
            out = []
            for inst in bb.instructions:
                si = inst.sync_info
                if si is not None and si.on_wait and len(si.on_wait) > max_waits:
                    for w in list(si.on_wait)[:-max_waits]:
                        out.append(mybir.InstNoOp(
                            name=f"waitsplit-{nc.next_id()}",
                            sync_info=mybir.SyncInfo(on_wait=[w], on_update=[]),
                            bass_nofuse=True,
                            engine=inst.engine,
                        ))
                        n += 1
                    si.on_wait = list(si.on_wait)[-max_waits:]
                out.append(inst)
            bb.instructions = out
    return n


def build_nc(n_tiles: int, iters: int = 1, split_waits: bool = True,
             tpg: int = TPG, xs_bufs: int = 2, work_bufs: int = 3,
             dve_t0_mod: int = 5, dve_t0_cnt: int = 5, pool_t0_cnt: int = 5):
    """One SPMD program for all 8 cores, n_tiles tiles of TOK tokens each.
    iters>1 repeats the whole computation (for wall-clock benchmarking)."""
    assert n_tiles % tpg == 0
    nst = n_tiles // tpg
    ntok = n_tiles * TOK

    nc = bass.Bass("TRN2", target_bir_lowering=False, debug=False)
    xs = nc.dram_tensor("xs", [ntok * TNUM, D], BF16, kind="ExternalInput").ap()
    xt = nc.dram_tensor("xt", [D, ntok], BF16, kind="ExternalInput").ap()
    ets = nc.dram_tensor("ets", [D, K], BF16, kind="ExternalInput").ap()
    lmap = nc.dram_tensor("lmap", [TOK, NG * TOK], BF16, kind="ExternalInput").ap()
    mkf = nc.dram_tensor("mkf", [128, n_tiles], F32, kind="ExternalInput").ap()
    g_out = nc.dram_tensor("g_out", [128, 2 * K], BF16, kind="ExternalOutput").ap()
    c_out = nc.dram_tensor("c_out", [1, K], F32, kind="ExternalOutput").ap()
    cc_out = nc.dram_tensor("cc_out", [128, 1], F32, kind="ExternalOutput").ap()

    # DRAM views. xs_v: (token,i)-interleaved partitions for the block-ones
    # matmul; xs_tok: token-major partitions (i in free dim) for the DVE
    # pairwise-tree tiles — same bytes, different access pattern.
    xs_v = xs.rearrange("(st jj g p) d -> st p jj g d", st=nst, jj=tpg, g=NG, p=TOK)
    xs_tok = xs.rearrange("(st jj p i) d -> st p jj i d", st=nst, jj=tpg, p=TOK, i=TNUM)
    xt_v = xt.rearrange("(dc p) (st tt) -> st p dc tt", dc=2, p=128, st=nst)
    ets_v = ets.rearrange("(dc p) k -> p dc k", dc=2)

    with tile.TileContext(nc) as tc:
        with tc.tile_pool(name="const", bufs=1) as constp, \
             tc.tile_pool(name="xsp", bufs=xs_bufs) as xsp, \
             tc.tile_pool(name="xtp", bufs=2) as xtp, \
             tc.tile_pool(name="work", bufs=work_bufs) as work, \
             tc.tile_pool(name="acc", bufs=1) as accp, \
             tc.tile_pool(name="gps", bufs=1, space="PSUM") as gps, \
             tc.tile_pool(name="cps", bufs=1, space="PSUM") as cps, \
             tc.tile_pool(name="tps", bufs=2, space="PSUM") as tps:

            # warm the ACT function tables while DMAs run
            warm = constp.tile([1, 1], F32, tag="warm")
            nc.vector.memset(warm[:], 0.0)
            warm2 = constp.tile([1, 1], BF16, tag="warm2")
            nc.scalar.activation(warm2[:], warm[:], ACTF.Copy)

            # constants on the ACT HWDGE ring; xs stream owns the SP ring
            ets_sb = constp.tile([128, 2, K], BF16, tag="ets")
            lmap_sb = constp.tile([TOK, NG * TOK], BF16, tag="lmap")
            mkf_sb = constp.tile([128, n_tiles], F32, tag="mkf")
            mkh_sb = constp.tile([128, n_tiles], BF16, tag="mkh")
            nc.scalar.dma_start(ets_sb[:], ets_v)
            if dve_t0_cnt < dve_t0_mod:  # lmap only used by PE-t0 tiles
                nc.scalar.dma_start(lmap_sb[:], lmap)
            nc.scalar.dma_start(mkf_sb[:], mkf)
            nc.vector.tensor_copy(mkh_sb[:], mkf_sb[:])  # bf16(0.1*v)

            for it in range(iters):
                g_ps = [[gps.tile([128, KC], F32, tag=f"g{dc}{kc}", name=f"g{dc}{kc}")
                         for kc in range(2)] for dc in range(2)]
                c_ps = [cps.tile([1, KC], F32, tag=f"c{kc}", name=f"c{kc}") for kc in range(2)]
                rc = accp.tile([128, n_tiles], F32, tag="rc")
                nc.vector.memset(rc[:], 0.0)

                for st in range(nst):
                    xs_sb = xsp.tile([TOK, tpg, NG, D], BF16, tag="xs")
                    xt_sb = xtp.tile([128, 2, tpg * TOK], BF16, tag="xt")
                    # per-tile loads: tree tiles get the token-major view,
                    # matmul tiles the (token,i)-interleaved view
                    for jj in range(tpg):
                        j = st * tpg + jj
                        view = (xs_tok if j % dve_t0_mod < dve_t0_cnt
                                else xs_v)
                        nc.sync.dma_start(xs_sb[:, jj:jj + 1],
                                          view[st][:, jj:jj + 1])
                        if st == 0 and it == 0 and jj == 0:
                            nc.sync.dma_start(xt_sb[:], xt_v[st])
                    if not (st == 0 and it == 0):
                        nc.scalar.dma_start(xt_sb[:], xt_v[st])

                    for jj in range(tpg):
                        j = st * tpg + jj
                        first = j == 0
                        last = j == n_tiles - 1

                        # score chunks -> psum, drained to SBUF by ACT fast
                        # so the transient psum slots never wait on DVE
                        s_ps = [tps.tile([TOK, KC], F32, tag="t", name=f"s{i}") for i in range(2)]
                        for kc in range(2):
                            for dc in range(2):
                                nc.tensor.matmul(
                                    s_ps[kc][:],
                                    xt_sb[:, dc, jj * TOK:(jj + 1) * TOK],
                                    ets_sb[:, dc, kc * KC:(kc + 1) * KC],
                                    start=(dc == 0), stop=(dc == 1))
                        sc_sb = work.tile([TOK, 2 * KC], F32, tag="sc")
                        for kc in range(2):
                            nc.scalar.activation(sc_sb[:, kc * KC:(kc + 1) * KC],
                                                 s_ps[kc][:], ACTF.Copy)

                        # t0 = sum_i xs[t, i, :]. PE (block-ones matmuls) is
                        # the busiest engine, so a slice of tiles computes it
                        # on DVE instead (bf16 pairwise tree runs at 2x mode)
                        # to balance engine load.
                        t0v = work.tile([TOK, D], BF16, tag="t0v")
                        if j % dve_t0_mod < dve_t0_cnt:
                            # a slice of tree tiles runs on the idle GpSimd
                            eng = (nc.gpsimd if j % dve_t0_mod < pool_t0_cnt
                                   else nc.vector)
                            tA = work.tile([TOK, 5, D], BF16, tag="tA")
                            eng.tensor_add(tA[:], xs_sb[:, jj, 0:5, :],
                                           xs_sb[:, jj, 5:10, :])
                            tB = work.tile([TOK, 2, D], BF16, tag="tB")
                            eng.tensor_add(tB[:], tA[:, 0:2, :],
                                           tA[:, 2:4, :])
                            tC = work.tile([TOK, D], BF16, tag="tC")
                            eng.tensor_add(tC[:], tB[:, 0, :], tB[:, 1, :])
                            tD = work.tile([TOK, D], BF16, tag="tD")
                            eng.tensor_add(tD[:], tC[:], tA[:, 4, :])
                            nc.vector.tensor_scalar(
                                t0v[:], tD[:], mkf_sb[0:TOK, j:j + 1], None,
                                op0=ALU.mult)
                        else:
                            t0_ps = tps.tile([TOK, D], F32, tag="t")
                            for g in range(NG):
                                nc.tensor.matmul(
                                    t0_ps[:],
                                    lmap_sb[:, g * TOK:(g + 1) * TOK],
                                    xs_sb[:, jj, g, :],
                                    start=(g == 0), stop=(g == NG - 1))
                            # t0v = (0.1 * v) * t0  (psum -> sbuf bf16)
                            nc.scalar.activation(t0v[:], t0_ps[:], ACTF.Copy,
                                                 scale=mkf_sb[0:TOK, j:j + 1])

                        # row max and onehot = (score == max) -> bf16, from SBUF
                        mm = work.tile([TOK, 1], F32, tag="mm")
                        nc.vector.tensor_reduce(mm[:], sc_sb[:],
                                                axis=mybir.AxisListType.X, op=ALU.max)
                        oh = work.tile([TOK, 2 * KC], BF16, tag="oh")
                        nc.vector.tensor_scalar(oh[:], sc_sb[:],
                                                mm[:, 0:1], None, op0=ALU.is_equal)

                        # G[dc][kc] += t0v.T-chunk x onehot ; c[kc] += v x onehot
                        for kc in range(2):
                            for dc in range(2):
                                nc.tensor.matmul(
                                    g_ps[dc][kc][:],
                                    t0v[:, dc * 128:(dc + 1) * 128],
                                    oh[:, kc * KC:(kc + 1) * KC],
                                    start=first, stop=last)
                            nc.tensor.matmul(
                                c_ps[kc][:],
                                mkh_sb[0:TOK, j:j + 1],
                                oh[:, kc * KC:(kc + 1) * KC],
                                start=first, stop=last)

                        # C: accumulate |t0v|^2 per partition into rc[:, j]
                        # (tree tiles put it on the Pool engine)
                        sq = work.tile([TOK, D], BF16, tag="sq")
                        nc.vector.scalar_tensor_tensor(
                            sq[:], t0v[:], 1.0, t0v[:],
                            op0=ALU.bypass, op1=ALU.mult,
                            accum_out=rc[0:TOK, j:j + 1])

                # outputs (overwritten every iter; identical data).
                # psum drains split across ACT and DVE; G shipped as bf16.
                g_sb = accp.tile([128, 2 * K], BF16, tag="gsb")
                for dc in range(2):
                    dst = g_sb[:, dc * K: dc * K + KC]
                    dst2 = g_sb[:, dc * K + KC: (dc + 1) * K]
                    nc.scalar.activation(dst, g_ps[dc][0][:], ACTF.Copy)
                    nc.vector.tensor_copy(dst2, g_ps[dc][1][:])
                    eng = nc.scalar if dc == 0 else nc.sync
                    eng.dma_start(g_out[:, dc * K:(dc + 1) * K],
                                  g_sb[:, dc * K:(dc + 1) * K])
                c_sb = accp.tile([1, K], F32, tag=f"csb_{it}")
                nc.scalar.activation(c_sb[:, 0:KC], c_ps[0][:], ACTF.Copy)
                nc.vector.tensor_copy(c_sb[:, KC:K], c_ps[1][:])
                cc_sb = accp.tile([128, 1], F32, tag=f"ccsb_{it}")
                nc.vector.tensor_reduce(cc_sb[:], rc[:],
                                        axis=mybir.AxisListType.X, op=ALU.add)
                nc.sync.dma_start(c_out, c_sb[:])
                nc.sync.dma_start(cc_out, cc_sb[:])

    if split_waits:
        _split_multi_waits(nc)
    return nc


def _build_lmap():
    lm = np.zeros((TOK, NG * TOK), dtype=BF)
    for g in range(NG):
        for p in range(TOK):
            lm[p, g * TOK + g * GRP + p // TNUM] = 1
    return lm


def pack_inputs(xs_pad_in, xs_pad_out, ilens, embed_w):
    ilens = np.asarray(ilens).astype(np.int64)
    ilens = np.minimum(np.maximum(ilens, 0), T)
    order = np.argsort(-ilens, kind="stable")
    assign = [[] for _ in range(NCORES)]
    loads = np.zeros(NCORES, dtype=np.int64)
    for bi in order:
        c = int(np.argmin(loads))
        assign[c].append(int(bi))
        loads[c] += int(ilens[bi])
    n_tiles = int(max(1, -(-int(loads.max()) // TOK)))
    n_tiles = -(-n_tiles // TPG) * TPG  # multiple of TPG (2) and of any tpg that divides it
    ntok = n_tiles * TOK

    inv = 1.0 / np.linalg.norm(np.asarray(embed_w, np.float32), axis=1)
    ets_np = (np.asarray(embed_w, np.float32) * inv[:, None]).T  # [D, K]
    ets_np = np.ascontiguousarray(ets_np).astype(BF)
    lmap_np = _build_lmap()

    xs_in = np.asarray(xs_pad_in, np.float32)
    xs_out = np.asarray(xs_pad_out, np.float32)

    in_maps = []
    for c in range(NCORES):
        nval = int(loads[c])
        xsb = np.zeros((ntok, TNUM, D), dtype=BF)
        xb = np.zeros((ntok, D), dtype=np.float32)
        pos = 0
        for bi in assign[c]:
            L = int(ilens[bi])
            if L > 0:
                xsb[pos:pos + L] = xs_out[bi, :L]
                xb[pos:pos + L] = xs_in[bi, :L]
                pos += L
        mk = np.zeros((128, n_tiles), dtype=np.float32)
        for j in range(n_tiles):
            lo = j * TOK
            n = min(max(nval - lo, 0), TOK)
            mk[:n, j] = float(MEAN)
        in_maps.append({
            "xs": xsb.reshape(ntok * TNUM, D),
            "xt": np.ascontiguousarray(xb.T).astype(BF),
            "ets": ets_np,
            "lmap": lmap_np,
            "mkf": mk,
        })
    return in_maps, n_tiles


_NC_CACHE = {}


def run_cores(in_maps, n_tiles, iters=1):
    key = (n_tiles, iters)
    if key not in _NC_CACHE:
        _NC_CACHE[key] = build_nc(n_tiles, iters)
    nc = _NC_CACHE[key]
    return run_bass_kernel_spmd(nc, in_maps, core_ids=list(range(NCORES)))


def combine(results, embed_w, W, b):
    E = np.asarray(embed_w, np.float64)
    Wf = np.asarray(W, np.float64)
    bf = np.asarray(b, np.float64)
    EWb = E @ Wf + bf                    # [K, D]
    s2 = (EWb * EWb).sum(axis=1)         # [K]
    sc = float(np.float32(MEAN).astype(BF))  # bf16(0.1) as used on device
    loss = 0.0
    for r in results:
        G = r["g_out"].astype(np.float64).reshape(128, 2, K)  # [p, dc, k]
        c = r["c_out"].astype(np.float64)[0] / sc             # counts
        C = r["cc_out"].astype(np.float64).sum()
        A = float(c @ s2)
        Bv = float((EWb[:, :128].T * G[:, 0, :]).sum()
                   + (EWb[:, 128:].T * G[:, 1, :]).sum())
        loss += 38.5 * A - 11.0 * Bv + C
    return np.asarray(loss, dtype=np.float32)


def kernel(xs_pad_in, xs_pad_out, ilens, ys_pad=None, embed_w=None, W=None, b=None):
    in_maps, n_tiles = pack_inputs(xs_pad_in, xs_pad_out, ilens, embed_w)
    res = run_cores(in_maps, n_tiles)
    return combine(res.results, embed_w, W, b)
